# revision 9
# baseline (speedup 1.0000x reference)
"""Self-contained Trainium2 Bass kernel: 16-head self-attention (B=4, N=2048,
C=1024, fp32), SPMD across 8 NeuronCores.

Entry point: kernel(**inputs) -> np.ndarray matching the reference module
(qkv projection + scaled-dot-product softmax attention + output projection).
See build_nc() docstring for the kernel design.
"""
import numpy as np

_NC_CACHE = {}


def kernel(x, Wqkv, bqkv, Wproj, bproj):
    from concourse.bass_utils import run_bass_kernel_spmd
    x = np.asarray(x, dtype=np.float32)
    Wqkv = np.asarray(Wqkv, dtype=np.float32)
    bqkv = np.asarray(bqkv, dtype=np.float32)
    Wproj = np.asarray(Wproj, dtype=np.float32)
    bproj = np.asarray(bproj, dtype=np.float32)
    # the rank-1 bias accumulation steps are emitted only when any bias is
    # actually nonzero (they are exact zeros in this problem's inputs)
    wb = bool(np.any(bqkv) or np.any(bproj))
    if wb not in _NC_CACHE:
        nc = build_nc(with_biases=wb)
        split_excess_waits(nc)
        _NC_CACHE[wb] = nc
    nc = _NC_CACHE[wb]
    in_maps = shard_inputs(x, Wqkv, bqkv, Wproj, bproj)
    res = run_bass_kernel_spmd(nc, in_maps, core_ids=list(range(N_CORES)))
    return unshard_output(res.results).astype(np.float32)


# ======================================================================
# IR post-pass: this walrus build accepts at most one semaphore wait per
# instruction; overflow waits move onto chained NoOps just before the
# instruction on the same engine queue.
# ======================================================================

# Walrus TPB_CTRL codegen (Drain/NoOp lowering) accepts only 1 sync wait;
# regular engine instructions accept more (tested empirically).
CTRL_OPCODES = {"Drain", "NoOp", "EventSemaphore", "AllEngineBarrier"}

def split_excess_waits(nc, engine_max=1, ctrl_max=1):
    n_split = 0
    for f in nc.m.functions:
        for bb in f.blocks:
            insts = list(bb.instructions)
            out = []
            changed = False
            for inst in insts:
                si = inst.sync_info
                max_w = ctrl_max if inst.opcode in CTRL_OPCODES else engine_max
                if si is not None and si.on_wait and len(si.on_wait) > max_w:
                    waits = list(si.on_wait)
                    extra, keep = waits[max_w:], waits[:max_w]
                    for i in range(0, len(extra), ctrl_max):
                        nop = bass_rust.InstNoOp(
                            name=f"{inst.name}-wsplit{i}", ins=[], outs=[])
                        nop.engine = inst.engine
                        nop.sync_info = mybir.SyncInfo(
                            on_wait=extra[i:i + ctrl_max], on_update=[])
                        out.append(nop)
                        n_split += 1
                    inst.sync_info = mybir.SyncInfo(
                        on_wait=keep, on_update=list(si.on_update))
                    changed = True
                out.append(inst)
            if changed:
                bb.instructions = out
    return n_split


# ======================================================================
# Kernel proper
# ======================================================================
import bass_rust
import concourse.bass as bass
import concourse.tile as tile
import concourse.mybir as mybir


F32 = mybir.dt.float32
BF16 = mybir.dt.bfloat16

N = 2048        # sequence length
C = 1024        # embed dim
HL = 8          # heads handled per core
D = 64          # head dim
SCALE = D ** -0.5
NHALF = N // 2
NQ = N // 4     # x tile quarter
VS = D + 1      # v columns per head incl. ones column
N_CORES = 8

AFT = mybir.ActivationFunctionType
ALU = mybir.AluOpType


def build_nc(with_biases=True):
    MDT = BF16
    nc = bass.Bass("TRN2", target_bir_lowering=False, debug=False,
                   num_devices=N_CORES)
    xt = nc.dram_tensor("xt", [C, N], MDT, kind="ExternalInput").ap()
    wq = nc.dram_tensor("wq", [C, HL * D], MDT, kind="ExternalInput").ap()
    wk = nc.dram_tensor("wk", [C, HL * D], MDT, kind="ExternalInput").ap()
    wv = nc.dram_tensor("wv", [C, HL * D], MDT, kind="ExternalInput").ap()
    wp = nc.dram_tensor("wp", [HL * D, C], MDT, kind="ExternalInput").ap()
    bqc = nc.dram_tensor("bqc", [128, 4], F32, kind="ExternalInput").ap()
    bkc = nc.dram_tensor("bkc", [128, 4], F32, kind="ExternalInput").ap()
    bv = nc.dram_tensor("bv", [1, HL * D], MDT, kind="ExternalInput").ap()
    bp = nc.dram_tensor("bp", [1, C], MDT, kind="ExternalInput").ap()
    ones_row = nc.dram_tensor("ones_row", [1, 512], MDT, kind="ExternalInput").ap()
    out = nc.dram_tensor("out", [N, C], F32, kind="ExternalOutput").ap()

    with tile.TileContext(nc) as tc:
        with tc.tile_pool(name="consts", bufs=1) as consts, \
             tc.tile_pool(name="persist", bufs=1) as persist, \
             tc.tile_pool(name="big", bufs=1) as bigp:

            # persistent activation tiles (bf16 so the attention matmuls
            # get full 128x128 stationary tiles + fast weight loads):
            #   qT: pair-packed [2 heads' d x 128, pair-group x n]
            #   kTp: per-head [128, head x n] with the partner head's 64
            #        partition rows zeroed (K=128 contraction, zeros kill
            #        the partner-q contribution in the shared qT rhs)
            #   v_sb: per (m-chunk, head) [128, 65]: cols 0:64 v, col 64
            #        ones (softmax denominator)
            qT = persist.tile([128, 4 * N], BF16, tag="qT")
            kTp = persist.tile([128, HL * N], BF16, tag="kTp")
            v_sb = persist.tile([128, 16 * HL * VS], BF16, tag="v")
            nc.gpsimd.memset(kTp, 0.0)
            vview = v_sb.rearrange("p (m h e) -> p m h e", h=HL, e=VS)
            nc.gpsimd.memset(vview[:, :, :, D:D + 1], 1.0)

            # ---------- Phase A: xT, qT/kT, v ----------
            # One consolidated dma_start per tensor; x in quarter tiles so
            # the first matmul group waits on 1MB, not 4MB.
            with tc.tile_pool(name="wpool", bufs=3) as wpool, \
                 tc.tile_pool(name="biases", bufs=1) as biasp, \
                 tc.tile_pool(name="xpool", bufs=4) as xpool, \
                 tc.tile_pool(name="kqp", bufs=2, space="PSUM") as kqp, \
                 tc.tile_pool(name="vpp", bufs=2, space="PSUM") as vpp:
                wq_sb = wpool.tile([128, 8 * 512], MDT, tag="w", name="wq_sb")
                wk_sb = wpool.tile([128, 8 * 512], MDT, tag="w", name="wk_sb")
                wv_sb = wpool.tile([128, 8 * 512], MDT, tag="w", name="wv_sb")
                xTq = [xpool.tile([128, 8 * NQ], MDT, tag="x", name=f"xT{i}")
                       for i in range(4)]

                def load_w(dst, src):
                    nc.sync.dma_start(
                        out=dst.rearrange("p (c d) -> p c d", d=512),
                        in_=src.rearrange("(c p) d -> p c d", p=128))

                load_w(wq_sb, wq)
                nc.sync.dma_start(
                    out=xTq[0].rearrange("p (c n) -> p c n", n=NQ),
                    in_=xt[:, 0:NQ].rearrange("(c p) n -> p c n", p=128))
                ones = consts.tile([1, 512], MDT, tag="ones")
                nc.sync.dma_start(out=ones, in_=ones_row)
                bqc_sb = biasp.tile([128, 4], F32, tag="bqc")
                bkc_sb = biasp.tile([128, 4], F32, tag="bkc")
                bv_sb = biasp.tile([1, HL * D], MDT, tag="bv")
                nc.sync.dma_start(out=bqc_sb, in_=bqc)
                nc.sync.dma_start(out=bkc_sb, in_=bkc)
                nc.sync.dma_start(out=bv_sb, in_=bv)
                nc.sync.dma_start(
                    out=xTq[1].rearrange("p (c n) -> p c n", n=NQ),
                    in_=xt[:, NQ:2 * NQ].rearrange("(c p) n -> p c n", p=128))
                load_w(wk_sb, wk)
                nc.sync.dma_start(
                    out=xTq[2].rearrange("p (c n) -> p c n", n=NQ),
                    in_=xt[:, 2 * NQ:3 * NQ].rearrange("(c p) n -> p c n", p=128))
                nc.sync.dma_start(
                    out=xTq[3].rearrange("p (c n) -> p c n", n=NQ),
                    in_=xt[:, 3 * NQ:4 * NQ].rearrange("(c p) n -> p c n", p=128))
                load_w(wv_sb, wv)

                # qT / kT (bias folded into evacuation)
                for dstT, w_sb, b_col in ((qT, wq_sb, bqc_sb),
                                          (kTp, wk_sb, bkc_sb)):
                    for nq in range(4):
                        xT = xTq[nq]
                        for g in range(4):
                            ps = kqp.tile([128, 512], F32, tag="kq",
                                          name=f"kq{nq}_{g}")
                            for c in range(8):
                                nc.tensor.matmul(
                                    ps,
                                    w_sb[:, c * 512 + g * 128:
                                         c * 512 + (g + 1) * 128],
                                    xT[:, c * NQ:c * NQ + 512],
                                    start=(c == 0), stop=(c == 7))
                            n0 = nq * NQ
                            if dstT is qT:
                                nc.vector.tensor_scalar_add(
                                    qT[:, g * N + n0: g * N + n0 + 512],
                                    ps, b_col[:, g:g + 1])
                            else:
                                for hh in range(2):
                                    h_, r0_ = 2 * g + hh, hh * D
                                    nc.vector.tensor_scalar_add(
                                        kTp[r0_:r0_ + D,
                                            h_ * N + n0: h_ * N + n0 + 512],
                                        ps[r0_:r0_ + D, :],
                                        b_col[r0_:r0_ + D, g:g + 1])
                # v
                for nq in range(4):
                    xT = xTq[nq]
                    for ml in range(NQ // 128):
                        mc = nq * (NQ // 128) + ml
                        ps = vpp.tile([128, 512], F32, tag="v",
                                      name=f"v{nq}_{ml}")
                        for c in range(8):
                            nc.tensor.matmul(
                                ps,
                                xT[:, c * NQ + ml * 128:
                                   c * NQ + (ml + 1) * 128],
                                wv_sb[:, c * 512:(c + 1) * 512],
                                start=(c == 0),
                                stop=(c == 7 and not with_biases))
                        if with_biases:
                            nc.tensor.matmul(ps, ones[0:1, 0:128],
                                             bv_sb[0:1, :],
                                             start=False, stop=True)
                        dst = v_sb[:, mc * HL * VS:(mc + 1) * HL * VS].rearrange(
                            "p (h e) -> p h e", e=VS)[:, :, 0:D]
                        nc.vector.tensor_copy(
                            dst, ps.rearrange("p (h e) -> p h e", e=D))

            # attn_outT (bf16, 16KB/partition)
            aoT = bigp.tile([128, 4 * N], MDT, tag="big", name="aoT")

            # ---------- Phase B + C: flat attention stream + proj ----------
            # Flat stream over (head-unit u = nh*8+h, m-chunk mcc): per chunk
            # emit [av(u, mcc - lag), sc(u, mcc), exp(u, mcc)].  The av lag
            # staircase (lag 3 for mcc 0/1, else 2) gives the previous head's
            # avs evacuation 2 chunks of slack before its PSUM slot is
            # reused, so the in-order PE queue never stalls on the DVE.
            # Softmax normalization: denominator row (av row 64) -> [128,8]
            # via sbuf-to-sbuf DMA -> DVE reciprocal -> partition-broadcast
            # DMA to [64,1024] -> one DVE multiply into aoT.  No PE involved.
            # Proj units (one output n-chunk nch x 512 cols) are injected at
            # head boundaries once their source half's aoT is complete.
            with tc.tile_pool(name="wppool", bufs=1) as wppool, \
                 tc.tile_pool(name="expp", bufs=4) as expp, \
                 tc.tile_pool(name="avsp", bufs=2) as avsp, \
                 tc.tile_pool(name="denp", bufs=2) as denp, \
                 tc.tile_pool(name="bcp", bufs=2) as bcp, \
                 tc.tile_pool(name="bpp", bufs=1) as bpp, \
                 tc.tile_pool(name="pout", bufs=3) as pout:
                wp_sb = wppool.tile([128, 4 * C], MDT, tag="wp", name="wp_sb")
                nc.sync.dma_start(
                    out=wp_sb.rearrange("p (g c) -> p g c", c=C),
                    in_=wp.rearrange("(g p) c -> p g c", p=128))
                bp_sb = bpp.tile([1, C], MDT, tag="bp")
                nc.sync.dma_start(out=bp_sb, in_=bp)

                with tc.tile_pool(name="scp", bufs=2, space="PSUM") as scp, \
                     tc.tile_pool(name="avp", bufs=1, space="PSUM") as avp, \
                     tc.tile_pool(name="pjp", bufs=2, space="PSUM") as pjp:

                    tails = []          # deferred normalization multiplies
                    proj_pending = []   # proj closures for a finished half
                    po_cur = {}         # nch -> pout tile awaiting 2nd jg

                    def emit_proj(nch, jg):
                        ps = pjp.tile([128, 512], F32, tag="pj",
                                      name=f"pj{nch}_{jg}")
                        for g in range(4):
                            nc.tensor.matmul(
                                ps,
                                aoT[:, g * N + nch * 128:
                                    g * N + (nch + 1) * 128],
                                wp_sb[:, g * C + jg * 512:
                                      g * C + jg * 512 + 512],
                                start=(g == 0),
                                stop=(g == 3 and not with_biases))
                        if with_biases:
                            nc.tensor.matmul(
                                ps, ones[0:1, 0:128],
                                bp_sb[0:1, jg * 512:(jg + 1) * 512],
                                start=False, stop=True)
                        if nch not in po_cur:
                            po_cur[nch] = pout.tile([128, C], F32, tag="po",
                                                    name=f"po{nch}")
                        po = po_cur[nch]
                        nc.vector.tensor_copy(
                            po[:, jg * 512:(jg + 1) * 512], ps)
                        if jg == 1:
                            nc.sync.dma_start(
                                out=out[nch * 128:(nch + 1) * 128, :],
                                in_=po)
                            del po_cur[nch]

                    # per-head state
                    av_t = [None] * 16
                    ex_t = [[None] * 16 for _ in range(16)]

                    def emit_sc_exp(u, mcc):
                        nh, h = divmod(u, HL)
                        g, n0 = h // 2, nh * NHALF
                        sc = scp.tile([128, NHALF], F32, tag="sc",
                                      name=f"sc{u}_{mcc}")
                        for ngl in range(2):
                            nc.tensor.matmul(
                                sc[:, ngl * 512:(ngl + 1) * 512],
                                kTp[:, h * N + mcc * 128:
                                    h * N + (mcc + 1) * 128],
                                qT[:, g * N + n0 + ngl * 512:
                                   g * N + n0 + (ngl + 1) * 512],
                                start=True, stop=True)
                        ex = expp.tile([128, NHALF], BF16, tag="ex",
                                       name=f"ex{u}_{mcc}")
                        nc.scalar.activation(ex, sc, AFT.Exp, scale=SCALE)
                        ex_t[u][mcc] = ex

                    def emit_av(u, mcc):
                        h = u % HL
                        if mcc == 0:
                            av_t[u] = avp.tile([VS, NHALF], F32, tag="av",
                                               name=f"av{u}")
                        av = av_t[u]
                        ex = ex_t[u][mcc]
                        ex_t[u][mcc] = None
                        for ngl in range(2):
                            nc.tensor.matmul(
                                av[:, ngl * 512:(ngl + 1) * 512],
                                v_sb[:, (mcc * HL + h) * VS:
                                     (mcc * HL + h + 1) * VS],
                                ex[:, ngl * 512:(ngl + 1) * 512],
                                start=(mcc == 0), stop=(mcc == 15))

                    recips = []         # deferred reciprocal chains

                    def head_finish(u):
                        """avs evacuation for head u.  The reciprocal chain
                        is deferred a few chunks (so the DVE reciprocal's
                        den-DMA input has landed before it hits the in-order
                        DVE queue); the normalization multiply is deferred a
                        full head."""
                        nh, h = divmod(u, HL)
                        g, r0, n0 = h // 2, (h % 2) * D, nh * NHALF
                        av = av_t[u]
                        avs = avsp.tile([VS, NHALF], MDT, tag="avs",
                                        name=f"avs{u}")
                        nc.vector.tensor_copy(avs, av[0:VS, :])
                        den = denp.tile([128, NHALF // 128], MDT,
                                        tag="den", name=f"den{u}")
                        nc.gpsimd.dma_start(out=den, in_=avs[D:VS, :])
                        bcast = bcp.tile([D, NHALF], MDT, tag="bc",
                                         name=f"bc{u}")

                        def recip_chain():
                            rcp = denp.tile([128, NHALF // 128], MDT,
                                            tag="rcp", name=f"rcp{u}")
                            with nc.allow_low_precision(reason="softmax den"):
                                nc.vector.reciprocal(rcp, den)
                            rrow = denp.tile([1, NHALF], MDT, tag="rrow",
                                             name=f"rrow{u}")
                            nc.gpsimd.dma_start(out=rrow, in_=rcp)
                            # replicate the reciprocal row across 64
                            # partitions (64 x 2KB descriptors) so the
                            # normalization is one DVE multiply -- no PE
                            # broadcast matmul needed
                            nc.gpsimd.dma_start(
                                out=bcast,
                                in_=rrow.unsqueeze(1).broadcast_to(
                                    [1, D, NHALF]))
                        recips.append(recip_chain)

                        def tail():
                            nc.vector.tensor_mul(
                                aoT[r0:r0 + D,
                                    g * N + n0: g * N + n0 + NHALF],
                                avs[0:D, :], bcast)
                        tails.append(tail)
                        if len(tails) > 1:
                            tails.pop(0)()

                    # av emission schedule per head chunk hc (0..15):
                    #   hc 3 -> av mcc 0 and 1; hc >= 4 -> av mcc hc-2
                    # (av for mcc 14/15 of head u run at hc 0/1 of head u+1)
                    def av_due(u, hc):
                        due = []
                        if hc == 0 and u > 0:
                            due.append((u - 1, 14))
                        elif hc == 1 and u > 0:
                            due.append((u - 1, 15))
                        elif hc == 3:
                            due.extend([(u, 0), (u, 1)])
                        elif hc >= 4:
                            due.append((u, hc - 2))
                        return due

                    for u in range(16):
                        for hc in range(16):
                            for (ua, mcc) in av_due(u, hc):
                                emit_av(ua, mcc)
                                if mcc == 15:
                                    head_finish(ua)
                                    # inject proj work for the finished half
                                    # (from u=10 so the last tail multiply of
                                    # that half is a full head ahead)
                                    if u >= 10:
                                        for _ in range(6):
                                            if proj_pending:
                                                proj_pending.pop(0)()
                            if hc == 5 and recips:
                                recips.pop(0)()
                            emit_sc_exp(u, hc)
                        if u == 7:
                            for nch in range(8):
                                for jg in range(2):
                                    proj_pending.append(
                                        lambda nch=nch, jg=jg:
                                            emit_proj(nch, jg))
                    # flush: remaining av chunks + last head's chain
                    emit_av(15, 14)
                    emit_av(15, 15)
                    head_finish(15)
                    while recips:
                        recips.pop(0)()
                    while tails:
                        tails.pop(0)()
                    while proj_pending:
                        proj_pending.pop(0)()
                    for nch in range(8, 16):
                        for jg in range(2):
                            emit_proj(nch, jg)
    return nc


def _bf16(a):
    import ml_dtypes
    return np.ascontiguousarray(a).astype(ml_dtypes.bfloat16)


def shard_inputs(x, Wqkv, bqkv, Wproj, bproj):
    """Full inputs -> per-core in_maps. Core c: batch c//2, head-group c%2."""
    in_maps = []
    for core in range(N_CORES):
        b, hg = core // 2, core % 2
        s = hg * 512
        m = {
            "xt": _bf16(x[b].T),
            "wq": _bf16(Wqkv[:, s:s + 512]),
            "wk": _bf16(Wqkv[:, C + s: C + s + 512]),
            "wv": _bf16(Wqkv[:, 2 * C + s: 2 * C + s + 512]),
            "wp": _bf16(Wproj[s:s + 512, :]),
            "bqc": np.ascontiguousarray(bqkv[s:s + 512].reshape(4, 128).T),
            "bkc": np.ascontiguousarray(bqkv[C + s: C + s + 512].reshape(4, 128).T),
            "bv": _bf16(bqkv[2 * C + s: 2 * C + s + 512][None, :]),
            "bp": _bf16(
                (bproj if hg == 0 else np.zeros_like(bproj))[None, :]),
            "ones_row": _bf16(np.ones((1, 512), np.float32)),
        }
        in_maps.append(m)
    return in_maps


def unshard_output(results):
    """Per-core partial outputs -> full [4, N, C]."""
    outs = []
    for b in range(4):
        outs.append(results[2 * b]["out"] + results[2 * b + 1]["out"])
    return np.stack(outs, axis=0)


# revision 15
# speedup vs baseline: 1.0220x; 1.0220x over previous
"""Self-contained Trainium2 Bass kernel: 16-head self-attention (B=4, N=2048,
C=1024, fp32), SPMD across 8 NeuronCores.

Entry point: kernel(**inputs) -> np.ndarray matching the reference module
(qkv projection + scaled-dot-product softmax attention + output projection).
See build_nc() docstring for the kernel design.
"""
import numpy as np

_NC_CACHE = {}


def kernel(x, Wqkv, bqkv, Wproj, bproj):
    from concourse.bass_utils import run_bass_kernel_spmd
    x = np.asarray(x, dtype=np.float32)
    Wqkv = np.asarray(Wqkv, dtype=np.float32)
    bqkv = np.asarray(bqkv, dtype=np.float32)
    Wproj = np.asarray(Wproj, dtype=np.float32)
    bproj = np.asarray(bproj, dtype=np.float32)
    # the rank-1 bias accumulation steps are emitted only when any bias is
    # actually nonzero (they are exact zeros in this problem's inputs)
    wb = bool(np.any(bqkv) or np.any(bproj))
    if wb not in _NC_CACHE:
        nc = build_nc(with_biases=wb)
        split_excess_waits(nc)
        _NC_CACHE[wb] = nc
    nc = _NC_CACHE[wb]
    in_maps = shard_inputs(x, Wqkv, bqkv, Wproj, bproj)
    res = run_bass_kernel_spmd(nc, in_maps, core_ids=list(range(N_CORES)))
    return unshard_output(res.results).astype(np.float32)


# ======================================================================
# IR post-pass: this walrus build accepts at most one semaphore wait per
# instruction; overflow waits move onto chained NoOps just before the
# instruction on the same engine queue.
# ======================================================================

# Walrus TPB_CTRL codegen (Drain/NoOp lowering) accepts only 1 sync wait;
# regular engine instructions accept more (tested empirically).
CTRL_OPCODES = {"Drain", "NoOp", "EventSemaphore", "AllEngineBarrier"}

def split_excess_waits(nc, engine_max=1, ctrl_max=1):
    n_split = 0
    for f in nc.m.functions:
        for bb in f.blocks:
            insts = list(bb.instructions)
            out = []
            changed = False
            for inst in insts:
                si = inst.sync_info
                max_w = ctrl_max if inst.opcode in CTRL_OPCODES else engine_max
                if si is not None and si.on_wait and len(si.on_wait) > max_w:
                    waits = list(si.on_wait)
                    extra, keep = waits[max_w:], waits[:max_w]
                    for i in range(0, len(extra), ctrl_max):
                        nop = bass_rust.InstNoOp(
                            name=f"{inst.name}-wsplit{i}", ins=[], outs=[])
                        nop.engine = inst.engine
                        nop.sync_info = mybir.SyncInfo(
                            on_wait=extra[i:i + ctrl_max], on_update=[])
                        out.append(nop)
                        n_split += 1
                    inst.sync_info = mybir.SyncInfo(
                        on_wait=keep, on_update=list(si.on_update))
                    changed = True
                out.append(inst)
            if changed:
                bb.instructions = out
    return n_split


# ======================================================================
# Kernel proper
# ======================================================================
import bass_rust
import concourse.bass as bass
import concourse.tile as tile
import concourse.mybir as mybir


F32 = mybir.dt.float32
BF16 = mybir.dt.bfloat16

N = 2048        # sequence length
C = 1024        # embed dim
HL = 8          # heads handled per core
D = 64          # head dim
SCALE = D ** -0.5
NHALF = N // 2
NQ = N // 4     # x tile quarter
VS = D + 1      # v columns per head incl. ones column
N_CORES = 8

AFT = mybir.ActivationFunctionType
ALU = mybir.AluOpType


def build_nc(with_biases=True):
    MDT = BF16
    nc = bass.Bass("TRN2", target_bir_lowering=False, debug=False,
                   num_devices=N_CORES)
    xt = nc.dram_tensor("xt", [C, N], MDT, kind="ExternalInput").ap()
    wq = nc.dram_tensor("wq", [C, HL * D], MDT, kind="ExternalInput").ap()
    wk = nc.dram_tensor("wk", [C, HL * D], MDT, kind="ExternalInput").ap()
    wv = nc.dram_tensor("wv", [C, HL * D], MDT, kind="ExternalInput").ap()
    wp = nc.dram_tensor("wp", [HL * D, C], MDT, kind="ExternalInput").ap()
    bqc = nc.dram_tensor("bqc", [128, 4], F32, kind="ExternalInput").ap()
    bkc = nc.dram_tensor("bkc", [128, 4], F32, kind="ExternalInput").ap()
    bv = nc.dram_tensor("bv", [1, HL * D], MDT, kind="ExternalInput").ap()
    bp = nc.dram_tensor("bp", [1, C], MDT, kind="ExternalInput").ap()
    ones_row = nc.dram_tensor("ones_row", [1, 512], MDT, kind="ExternalInput").ap()
    out = nc.dram_tensor("out", [N, C], F32, kind="ExternalOutput").ap()

    with tile.TileContext(nc) as tc:
        with tc.tile_pool(name="consts", bufs=1) as consts, \
             tc.tile_pool(name="persist", bufs=1) as persist, \
             tc.tile_pool(name="big", bufs=1) as bigp:

            # persistent activation tiles (bf16 so the attention matmuls
            # get full 128x128 stationary tiles + fast weight loads):
            #   qT: pair-packed [2 heads' d x 128, pair-group x n]
            #   kTp: per-head [128, head x n] with the partner head's 64
            #        partition rows zeroed (K=128 contraction, zeros kill
            #        the partner-q contribution in the shared qT rhs)
            #   v_sb: per (m-chunk, head) [128, 65]: cols 0:64 v, col 64
            #        ones (softmax denominator)
            qT = persist.tile([128, 4 * N], BF16, tag="qT")
            kTp = persist.tile([128, HL * N], BF16, tag="kTp")
            v_sb = persist.tile([128, 16 * HL * VS], BF16, tag="v")
            nc.gpsimd.memset(kTp, 0.0)
            vview = v_sb.rearrange("p (m h e) -> p m h e", h=HL, e=VS)
            nc.gpsimd.memset(vview[:, :, :, D:D + 1], 1.0)

            # ---------- Phase A: xT, qT/kT, v ----------
            # One consolidated dma_start per tensor; x in quarter tiles so
            # the first matmul group waits on 1MB, not 4MB.
            with tc.tile_pool(name="wpool", bufs=3) as wpool, \
                 tc.tile_pool(name="biases", bufs=1) as biasp, \
                 tc.tile_pool(name="xpool", bufs=4) as xpool, \
                 tc.tile_pool(name="kqp", bufs=2, space="PSUM") as kqp, \
                 tc.tile_pool(name="vpp", bufs=2, space="PSUM") as vpp:
                # wq and x quarter 0 are split into half-tiles (c-chunks 0-3
                # / 4-7) so the very first matmuls wait on 1MB, not 2MB
                wq_sbh = [wpool.tile([128, 4 * 512], MDT, tag="wh",
                                     name=f"wq_sb{i}") for i in range(2)]
                wk_sb = wpool.tile([128, 8 * 512], MDT, tag="w", name="wk_sb")
                wv_sb = wpool.tile([128, 8 * 512], MDT, tag="w", name="wv_sb")
                xT0h = [xpool.tile([128, 4 * NQ], MDT, tag="xh",
                                   name=f"xT0{i}") for i in range(2)]
                xTq = [None] + [xpool.tile([128, 8 * NQ], MDT, tag="x",
                                           name=f"xT{i}") for i in range(1, 4)]

                def wq_slice(c, g):
                    return wq_sbh[c // 4][:, (c % 4) * 512 + g * 128:
                                          (c % 4) * 512 + (g + 1) * 128]

                def wk_slice(c, g):
                    return wk_sb[:, c * 512 + g * 128:c * 512 + (g + 1) * 128]

                def x_slice(nq, c, lo, hi):
                    if nq == 0:
                        return xT0h[c // 4][:, (c % 4) * NQ + lo:
                                            (c % 4) * NQ + hi]
                    return xTq[nq][:, c * NQ + lo:c * NQ + hi]

                def load_w(dst, src):
                    nc.sync.dma_start(
                        out=dst.rearrange("p (c d) -> p c d", d=512),
                        in_=src.rearrange("(c p) d -> p c d", p=128))

                load_w(wq_sbh[0], wq[0:512, :])
                nc.sync.dma_start(
                    out=xT0h[0].rearrange("p (c n) -> p c n", n=NQ),
                    in_=xt[0:512, 0:NQ].rearrange("(c p) n -> p c n", p=128))
                load_w(wq_sbh[1], wq[512:1024, :])
                nc.sync.dma_start(
                    out=xT0h[1].rearrange("p (c n) -> p c n", n=NQ),
                    in_=xt[512:1024, 0:NQ].rearrange("(c p) n -> p c n", p=128))
                ones = consts.tile([1, 512], MDT, tag="ones")
                nc.sync.dma_start(out=ones, in_=ones_row)
                bqc_sb = biasp.tile([128, 4], F32, tag="bqc")
                bkc_sb = biasp.tile([128, 4], F32, tag="bkc")
                bv_sb = biasp.tile([1, HL * D], MDT, tag="bv")
                nc.sync.dma_start(out=bqc_sb, in_=bqc)
                nc.sync.dma_start(out=bkc_sb, in_=bkc)
                nc.sync.dma_start(out=bv_sb, in_=bv)
                nc.sync.dma_start(
                    out=xTq[1].rearrange("p (c n) -> p c n", n=NQ),
                    in_=xt[:, NQ:2 * NQ].rearrange("(c p) n -> p c n", p=128))
                load_w(wk_sb, wk)
                nc.sync.dma_start(
                    out=xTq[2].rearrange("p (c n) -> p c n", n=NQ),
                    in_=xt[:, 2 * NQ:3 * NQ].rearrange("(c p) n -> p c n", p=128))
                nc.sync.dma_start(
                    out=xTq[3].rearrange("p (c n) -> p c n", n=NQ),
                    in_=xt[:, 3 * NQ:4 * NQ].rearrange("(c p) n -> p c n", p=128))
                load_w(wv_sb, wv)

                # qT / kT (bias folded into evacuation)
                for dstT, w_slice, b_col in ((qT, wq_slice, bqc_sb),
                                             (kTp, wk_slice, bkc_sb)):
                    for nq in range(4):
                        for g in range(4):
                            ps = kqp.tile([128, 512], F32, tag="kq",
                                          name=f"kq{nq}_{g}")
                            for c in range(8):
                                nc.tensor.matmul(
                                    ps,
                                    w_slice(c, g),
                                    x_slice(nq, c, 0, 512),
                                    start=(c == 0), stop=(c == 7))
                            n0 = nq * NQ
                            if dstT is qT:
                                nc.vector.tensor_scalar_add(
                                    qT[:, g * N + n0: g * N + n0 + 512],
                                    ps, b_col[:, g:g + 1])
                            else:
                                for hh in range(2):
                                    h_, r0_ = 2 * g + hh, hh * D
                                    nc.vector.tensor_scalar_add(
                                        kTp[r0_:r0_ + D,
                                            h_ * N + n0: h_ * N + n0 + 512],
                                        ps[r0_:r0_ + D, :],
                                        b_col[r0_:r0_ + D, g:g + 1])
                # v
                for nq in range(4):
                    for ml in range(NQ // 128):
                        mc = nq * (NQ // 128) + ml
                        ps = vpp.tile([128, 512], F32, tag="v",
                                      name=f"v{nq}_{ml}")
                        for c in range(8):
                            nc.tensor.matmul(
                                ps,
                                x_slice(nq, c, ml * 128, (ml + 1) * 128),
                                wv_sb[:, c * 512:(c + 1) * 512],
                                start=(c == 0),
                                stop=(c == 7 and not with_biases))
                        if with_biases:
                            nc.tensor.matmul(ps, ones[0:1, 0:128],
                                             bv_sb[0:1, :],
                                             start=False, stop=True)
                        dst = v_sb[:, mc * HL * VS:(mc + 1) * HL * VS].rearrange(
                            "p (h e) -> p h e", e=VS)[:, :, 0:D]
                        nc.vector.tensor_copy(
                            dst, ps.rearrange("p (h e) -> p h e", e=D))

            # attn_outT (bf16, 16KB/partition)
            aoT = bigp.tile([128, 4 * N], MDT, tag="big", name="aoT")

            # ---------- Phase B + C: flat attention stream + proj ----------
            # Flat stream over (head-unit u = nh*8+h, m-chunk mcc): per chunk
            # emit [av(u, mcc - lag), sc(u, mcc), exp(u, mcc)].  The av lag
            # staircase (lag 3 for mcc 0/1, else 2) gives the previous head's
            # avs evacuation 2 chunks of slack before its PSUM slot is
            # reused, so the in-order PE queue never stalls on the DVE.
            # Softmax normalization: denominator row (av row 64) -> [128,8]
            # via sbuf-to-sbuf DMA -> DVE reciprocal -> partition-broadcast
            # DMA to [64,1024] -> one DVE multiply into aoT.  No PE involved.
            # Proj units (one output n-chunk nch x 512 cols) are injected at
            # head boundaries once their source half's aoT is complete.
            with tc.tile_pool(name="wppool", bufs=1) as wppool, \
                 tc.tile_pool(name="expp", bufs=4) as expp, \
                 tc.tile_pool(name="avsp", bufs=4) as avsp, \
                 tc.tile_pool(name="denp", bufs=4) as denp, \
                 tc.tile_pool(name="bcp", bufs=3) as bcp, \
                 tc.tile_pool(name="bpp", bufs=1) as bpp, \
                 tc.tile_pool(name="pout", bufs=3) as pout:
                wp_sb = wppool.tile([128, 4 * C], MDT, tag="wp", name="wp_sb")
                nc.sync.dma_start(
                    out=wp_sb.rearrange("p (g c) -> p g c", c=C),
                    in_=wp.rearrange("(g p) c -> p g c", p=128))
                bp_sb = bpp.tile([1, C], MDT, tag="bp")
                nc.sync.dma_start(out=bp_sb, in_=bp)

                with tc.tile_pool(name="scp", bufs=2, space="PSUM") as scp, \
                     tc.tile_pool(name="avp", bufs=1, space="PSUM") as avp, \
                     tc.tile_pool(name="pjp", bufs=2, space="PSUM") as pjp:

                    tails = []          # deferred normalization multiplies
                    proj_pending = []   # proj closures for a finished half
                    po_cur = {}         # nch -> pout tile awaiting 2nd jg

                    def emit_proj(nch, jg):
                        ps = pjp.tile([128, 512], F32, tag="pj",
                                      name=f"pj{nch}_{jg}")
                        for g in range(4):
                            nc.tensor.matmul(
                                ps,
                                aoT[:, g * N + nch * 128:
                                    g * N + (nch + 1) * 128],
                                wp_sb[:, g * C + jg * 512:
                                      g * C + jg * 512 + 512],
                                start=(g == 0),
                                stop=(g == 3 and not with_biases))
                        if with_biases:
                            nc.tensor.matmul(
                                ps, ones[0:1, 0:128],
                                bp_sb[0:1, jg * 512:(jg + 1) * 512],
                                start=False, stop=True)
                        if nch not in po_cur:
                            po_cur[nch] = pout.tile([128, C], F32, tag="po",
                                                    name=f"po{nch}")
                        po = po_cur[nch]
                        nc.vector.tensor_copy(
                            po[:, jg * 512:(jg + 1) * 512], ps)
                        if jg == 1:
                            nc.sync.dma_start(
                                out=out[nch * 128:(nch + 1) * 128, :],
                                in_=po)
                            del po_cur[nch]

                    # per-head state
                    av_t = [None] * 16
                    ex_t = [[None] * 16 for _ in range(16)]

                    def emit_sc_exp(u, mcc):
                        nh, h = divmod(u, HL)
                        g, n0 = h // 2, nh * NHALF
                        sc = scp.tile([128, NHALF], F32, tag="sc",
                                      name=f"sc{u}_{mcc}")
                        for ngl in range(2):
                            nc.tensor.matmul(
                                sc[:, ngl * 512:(ngl + 1) * 512],
                                kTp[:, h * N + mcc * 128:
                                    h * N + (mcc + 1) * 128],
                                qT[:, g * N + n0 + ngl * 512:
                                   g * N + n0 + (ngl + 1) * 512],
                                start=True, stop=True)
                        ex = expp.tile([128, NHALF], BF16, tag="ex",
                                       name=f"ex{u}_{mcc}")
                        nc.scalar.activation(ex, sc, AFT.Exp, scale=SCALE)
                        ex_t[u][mcc] = ex

                    def emit_av(u, mcc):
                        h = u % HL
                        if mcc == 0:
                            av_t[u] = avp.tile([VS, NHALF], F32, tag="av",
                                               name=f"av{u}")
                        av = av_t[u]
                        ex = ex_t[u][mcc]
                        ex_t[u][mcc] = None
                        for ngl in range(2):
                            nc.tensor.matmul(
                                av[:, ngl * 512:(ngl + 1) * 512],
                                v_sb[:, (mcc * HL + h) * VS:
                                     (mcc * HL + h + 1) * VS],
                                ex[:, ngl * 512:(ngl + 1) * 512],
                                start=(mcc == 0), stop=(mcc == 15))

                    recips = []         # deferred reciprocal chains

                    def head_finish(u):
                        """avs evacuation for head u.  The reciprocal chain
                        is deferred a few chunks (so the DVE reciprocal's
                        den-DMA input has landed before it hits the in-order
                        DVE queue); the normalization multiply is deferred
                        three heads so its broadcast-DMA chain (~8us) never
                        delays the in-order DVE queue.  The very last head
                        uses a PE broadcast matmul instead of the broadcast
                        DMA to shorten the end-of-kernel serial chain."""
                        nh, h = divmod(u, HL)
                        g, r0, n0 = h // 2, (h % 2) * D, nh * NHALF
                        last = (u == 15)
                        av = av_t[u]
                        avs = avsp.tile([VS, NHALF], MDT, tag="avs",
                                        name=f"avs{u}")
                        nc.vector.tensor_copy(avs, av[0:VS, :])
                        den = denp.tile([128, NHALF // 128], MDT,
                                        tag="den", name=f"den{u}")
                        nc.gpsimd.dma_start(out=den, in_=avs[D:VS, :])
                        bcast = bcp.tile([D, NHALF], MDT, tag="bc",
                                         name=f"bc{u}")
                        rrow = denp.tile([1, NHALF], MDT, tag="rrow",
                                         name=f"rrow{u}")

                        def recip_chain():
                            rcp = denp.tile([128, NHALF // 128], MDT,
                                            tag="rcp", name=f"rcp{u}")
                            with nc.allow_low_precision(reason="softmax den"):
                                nc.vector.reciprocal(rcp, den)
                            nc.gpsimd.dma_start(out=rrow, in_=rcp)
                            if not last:
                                # replicate the reciprocal row across 64
                                # partitions (64 x 2KB descriptors) so the
                                # normalization is one DVE multiply -- no PE
                                # broadcast matmul needed
                                nc.gpsimd.dma_start(
                                    out=bcast,
                                    in_=rrow.unsqueeze(1).broadcast_to(
                                        [1, D, NHALF]))
                        recips.append(recip_chain)

                        def tail():
                            if last:
                                bc = scp.tile([D, NHALF], F32, tag="sc",
                                              name="bc15")
                                for ngl in range(2):
                                    nc.tensor.matmul(
                                        bc[:, ngl * 512:(ngl + 1) * 512],
                                        ones[0:1, 0:D],
                                        rrow[0:1, ngl * 512:(ngl + 1) * 512],
                                        start=True, stop=True)
                                src1 = bc
                            else:
                                src1 = bcast
                            nc.vector.tensor_mul(
                                aoT[r0:r0 + D,
                                    g * N + n0: g * N + n0 + NHALF],
                                avs[0:D, :], src1)
                        tails.append(tail)
                        if len(tails) > 2:
                            tails.pop(0)()

                    # av emission schedule per head chunk hc (0..15):
                    #   hc 3 -> av mcc 0 and 1; hc >= 4 -> av mcc hc-2
                    # (av for mcc 14/15 of head u run at hc 0/1 of head u+1)
                    def av_due(u, hc):
                        due = []
                        if hc == 0 and u > 0:
                            due.append((u - 1, 14))
                        elif hc == 1 and u > 0:
                            due.append((u - 1, 15))
                        elif hc == 3:
                            due.extend([(u, 0), (u, 1)])
                        elif hc >= 4:
                            due.append((u, hc - 2))
                        return due

                    for u in range(16):
                        for hc in range(16):
                            for (ua, mcc) in av_due(u, hc):
                                emit_av(ua, mcc)
                                if mcc == 15:
                                    head_finish(ua)
                                    # inject proj work for the finished half
                                    # (from u=11 so the last tail multiply of
                                    # that half is a full head ahead)
                                    if u >= 11:
                                        for _ in range(4):
                                            if proj_pending:
                                                proj_pending.pop(0)()
                            if hc == 5 and recips:
                                recips.pop(0)()
                            emit_sc_exp(u, hc)
                        if u == 7:
                            for nch in range(8):
                                for jg in range(2):
                                    proj_pending.append(
                                        lambda nch=nch, jg=jg:
                                            emit_proj(nch, jg))
                    # flush: remaining av chunks + last head's chain
                    emit_av(15, 14)
                    emit_av(15, 15)
                    head_finish(15)
                    while recips:
                        recips.pop(0)()
                    while tails:
                        tails.pop(0)()
                    while proj_pending:
                        proj_pending.pop(0)()
                    for nch in range(8, 16):
                        for jg in range(2):
                            emit_proj(nch, jg)
    return nc


def _bf16(a):
    import ml_dtypes
    return np.ascontiguousarray(a).astype(ml_dtypes.bfloat16)


def shard_inputs(x, Wqkv, bqkv, Wproj, bproj):
    """Full inputs -> per-core in_maps. Core c: batch c//2, head-group c%2."""
    in_maps = []
    for core in range(N_CORES):
        b, hg = core // 2, core % 2
        s = hg * 512
        m = {
            "xt": _bf16(x[b].T),
            "wq": _bf16(Wqkv[:, s:s + 512]),
            "wk": _bf16(Wqkv[:, C + s: C + s + 512]),
            "wv": _bf16(Wqkv[:, 2 * C + s: 2 * C + s + 512]),
            "wp": _bf16(Wproj[s:s + 512, :]),
            "bqc": np.ascontiguousarray(bqkv[s:s + 512].reshape(4, 128).T),
            "bkc": np.ascontiguousarray(bqkv[C + s: C + s + 512].reshape(4, 128).T),
            "bv": _bf16(bqkv[2 * C + s: 2 * C + s + 512][None, :]),
            "bp": _bf16(
                (bproj if hg == 0 else np.zeros_like(bproj))[None, :]),
            "ones_row": _bf16(np.ones((1, 512), np.float32)),
        }
        in_maps.append(m)
    return in_maps


def unshard_output(results):
    """Per-core partial outputs -> full [4, N, C]."""
    outs = []
    for b in range(4):
        outs.append(results[2 * b]["out"] + results[2 * b + 1]["out"])
    return np.stack(outs, axis=0)


# revision 16
# speedup vs baseline: 1.0679x; 1.0449x over previous
"""Self-contained Trainium2 Bass kernel: 16-head self-attention (B=4, N=2048,
C=1024, fp32), SPMD across 8 NeuronCores.

Entry point: kernel(**inputs) -> np.ndarray matching the reference module
(qkv projection + scaled-dot-product softmax attention + output projection).
See build_nc() docstring for the kernel design.
"""
import numpy as np

_NC_CACHE = {}


def kernel(x, Wqkv, bqkv, Wproj, bproj):
    from concourse.bass_utils import run_bass_kernel_spmd
    x = np.asarray(x, dtype=np.float32)
    Wqkv = np.asarray(Wqkv, dtype=np.float32)
    bqkv = np.asarray(bqkv, dtype=np.float32)
    Wproj = np.asarray(Wproj, dtype=np.float32)
    bproj = np.asarray(bproj, dtype=np.float32)
    # the rank-1 bias accumulation steps are emitted only when any bias is
    # actually nonzero (they are exact zeros in this problem's inputs)
    wb = bool(np.any(bqkv) or np.any(bproj))
    if wb not in _NC_CACHE:
        nc = build_nc(with_biases=wb)
        split_excess_waits(nc)
        _NC_CACHE[wb] = nc
    nc = _NC_CACHE[wb]
    in_maps = shard_inputs(x, Wqkv, bqkv, Wproj, bproj)
    res = run_bass_kernel_spmd(nc, in_maps, core_ids=list(range(N_CORES)))
    return unshard_output(res.results).astype(np.float32)


# ======================================================================
# IR post-pass: this walrus build accepts at most one semaphore wait per
# instruction; overflow waits move onto chained NoOps just before the
# instruction on the same engine queue.
# ======================================================================

# Walrus TPB_CTRL codegen (Drain/NoOp lowering) accepts only 1 sync wait;
# regular engine instructions accept more (tested empirically).
CTRL_OPCODES = {"Drain", "NoOp", "EventSemaphore", "AllEngineBarrier"}

def split_excess_waits(nc, engine_max=1, ctrl_max=1):
    n_split = 0
    for f in nc.m.functions:
        for bb in f.blocks:
            insts = list(bb.instructions)
            out = []
            changed = False
            for inst in insts:
                si = inst.sync_info
                max_w = ctrl_max if inst.opcode in CTRL_OPCODES else engine_max
                if si is not None and si.on_wait and len(si.on_wait) > max_w:
                    waits = list(si.on_wait)
                    extra, keep = waits[max_w:], waits[:max_w]
                    for i in range(0, len(extra), ctrl_max):
                        nop = bass_rust.InstNoOp(
                            name=f"{inst.name}-wsplit{i}", ins=[], outs=[])
                        nop.engine = inst.engine
                        nop.sync_info = mybir.SyncInfo(
                            on_wait=extra[i:i + ctrl_max], on_update=[])
                        out.append(nop)
                        n_split += 1
                    inst.sync_info = mybir.SyncInfo(
                        on_wait=keep, on_update=list(si.on_update))
                    changed = True
                out.append(inst)
            if changed:
                bb.instructions = out
    return n_split


# ======================================================================
# Kernel proper
# ======================================================================
import bass_rust
import concourse.bass as bass
import concourse.tile as tile
import concourse.mybir as mybir


F32 = mybir.dt.float32
BF16 = mybir.dt.bfloat16

N = 2048        # sequence length
C = 1024        # embed dim
HL = 8          # heads handled per core
D = 64          # head dim
SCALE = D ** -0.5
NHALF = N // 2
NQ = N // 4     # x tile quarter
VS = D + 1      # v columns per head incl. ones column
N_CORES = 8

AFT = mybir.ActivationFunctionType
ALU = mybir.AluOpType


def build_nc(with_biases=True):
    MDT = BF16
    nc = bass.Bass("TRN2", target_bir_lowering=False, debug=False,
                   num_devices=N_CORES)
    xt = nc.dram_tensor("xt", [C, N], MDT, kind="ExternalInput").ap()
    wq = nc.dram_tensor("wq", [C, HL * D], MDT, kind="ExternalInput").ap()
    wk = nc.dram_tensor("wk", [C, HL * D], MDT, kind="ExternalInput").ap()
    wv = nc.dram_tensor("wv", [C, HL * D], MDT, kind="ExternalInput").ap()
    wp = nc.dram_tensor("wp", [HL * D, C], MDT, kind="ExternalInput").ap()
    bqc = nc.dram_tensor("bqc", [128, 4], F32, kind="ExternalInput").ap()
    bkc = nc.dram_tensor("bkc", [128, 4], F32, kind="ExternalInput").ap()
    bv = nc.dram_tensor("bv", [1, HL * D], MDT, kind="ExternalInput").ap()
    bp = nc.dram_tensor("bp", [1, C], MDT, kind="ExternalInput").ap()
    ones_row = nc.dram_tensor("ones_row", [1, 512], MDT, kind="ExternalInput").ap()
    out = nc.dram_tensor("out", [N, C], F32, kind="ExternalOutput").ap()

    with tile.TileContext(nc) as tc:
        with tc.tile_pool(name="consts", bufs=1) as consts, \
             tc.tile_pool(name="persist", bufs=1) as persist, \
             tc.tile_pool(name="big", bufs=1) as bigp:

            # persistent activation tiles (bf16 so the attention matmuls
            # get full 128x128 stationary tiles + fast weight loads):
            #   qT: pair-packed [2 heads' d x 128, pair-group x n]
            #   kTp: per-head [128, head x n] with the partner head's 64
            #        partition rows zeroed (K=128 contraction, zeros kill
            #        the partner-q contribution in the shared qT rhs)
            #   v_sb: per (m-chunk, head) [128, 65]: cols 0:64 v, col 64
            #        ones (softmax denominator)
            qT = persist.tile([128, 4 * N], BF16, tag="qT")
            kTp = persist.tile([128, HL * N], BF16, tag="kTp")
            v_sb = persist.tile([128, 16 * HL * VS], BF16, tag="v")
            nc.gpsimd.memset(kTp, 0.0)
            vview = v_sb.rearrange("p (m h e) -> p m h e", h=HL, e=VS)
            nc.gpsimd.memset(vview[:, :, :, D:D + 1], 1.0)

            # ---------- Phase A: xT, qT/kT, v ----------
            # One consolidated dma_start per tensor; x in quarter tiles so
            # the first matmul group waits on 1MB, not 4MB.
            with tc.tile_pool(name="wpool", bufs=3) as wpool, \
                 tc.tile_pool(name="biases", bufs=1) as biasp, \
                 tc.tile_pool(name="xpool", bufs=4) as xpool, \
                 tc.tile_pool(name="kqp", bufs=2, space="PSUM") as kqp, \
                 tc.tile_pool(name="vpp", bufs=2, space="PSUM") as vpp:
                # wq and x quarter 0 are split into half-tiles (c-chunks 0-3
                # / 4-7) so the very first matmuls wait on 1MB, not 2MB
                wq_sbh = [wpool.tile([128, 4 * 512], MDT, tag="wh",
                                     name=f"wq_sb{i}") for i in range(2)]
                wk_sb = wpool.tile([128, 8 * 512], MDT, tag="w", name="wk_sb")
                wv_sb = wpool.tile([128, 8 * 512], MDT, tag="w", name="wv_sb")
                xT0h = [xpool.tile([128, 4 * NQ], MDT, tag="xh",
                                   name=f"xT0{i}") for i in range(2)]
                xTq = [None] + [xpool.tile([128, 8 * NQ], MDT, tag="x",
                                           name=f"xT{i}") for i in range(1, 4)]

                def wq_slice(c, g):
                    return wq_sbh[c // 4][:, (c % 4) * 512 + g * 128:
                                          (c % 4) * 512 + (g + 1) * 128]

                def wk_slice(c, g):
                    return wk_sb[:, c * 512 + g * 128:c * 512 + (g + 1) * 128]

                def x_slice(nq, c, lo, hi):
                    if nq == 0:
                        return xT0h[c // 4][:, (c % 4) * NQ + lo:
                                            (c % 4) * NQ + hi]
                    return xTq[nq][:, c * NQ + lo:c * NQ + hi]

                def load_w(dst, src):
                    nc.sync.dma_start(
                        out=dst.rearrange("p (c d) -> p c d", d=512),
                        in_=src.rearrange("(c p) d -> p c d", p=128))

                load_w(wq_sbh[0], wq[0:512, :])
                nc.sync.dma_start(
                    out=xT0h[0].rearrange("p (c n) -> p c n", n=NQ),
                    in_=xt[0:512, 0:NQ].rearrange("(c p) n -> p c n", p=128))
                load_w(wq_sbh[1], wq[512:1024, :])
                nc.sync.dma_start(
                    out=xT0h[1].rearrange("p (c n) -> p c n", n=NQ),
                    in_=xt[512:1024, 0:NQ].rearrange("(c p) n -> p c n", p=128))
                ones = consts.tile([1, 512], MDT, tag="ones")
                nc.sync.dma_start(out=ones, in_=ones_row)
                bqc_sb = biasp.tile([128, 4], F32, tag="bqc")
                bkc_sb = biasp.tile([128, 4], F32, tag="bkc")
                bv_sb = biasp.tile([1, HL * D], MDT, tag="bv")
                nc.sync.dma_start(out=bqc_sb, in_=bqc)
                nc.sync.dma_start(out=bkc_sb, in_=bkc)
                nc.sync.dma_start(out=bv_sb, in_=bv)
                nc.sync.dma_start(
                    out=xTq[1].rearrange("p (c n) -> p c n", n=NQ),
                    in_=xt[:, NQ:2 * NQ].rearrange("(c p) n -> p c n", p=128))
                load_w(wk_sb, wk)
                nc.sync.dma_start(
                    out=xTq[2].rearrange("p (c n) -> p c n", n=NQ),
                    in_=xt[:, 2 * NQ:3 * NQ].rearrange("(c p) n -> p c n", p=128))
                nc.sync.dma_start(
                    out=xTq[3].rearrange("p (c n) -> p c n", n=NQ),
                    in_=xt[:, 3 * NQ:4 * NQ].rearrange("(c p) n -> p c n", p=128))
                load_w(wv_sb, wv)

                # qT / kT (bias folded into evacuation)
                for dstT, w_slice, b_col in ((qT, wq_slice, bqc_sb),
                                             (kTp, wk_slice, bkc_sb)):
                    for nq in range(4):
                        for g in range(4):
                            ps = kqp.tile([128, 512], F32, tag="kq",
                                          name=f"kq{nq}_{g}")
                            for c in range(8):
                                nc.tensor.matmul(
                                    ps,
                                    w_slice(c, g),
                                    x_slice(nq, c, 0, 512),
                                    start=(c == 0), stop=(c == 7))
                            n0 = nq * NQ
                            if dstT is qT:
                                nc.vector.tensor_scalar_add(
                                    qT[:, g * N + n0: g * N + n0 + 512],
                                    ps, b_col[:, g:g + 1])
                            else:
                                for hh in range(2):
                                    h_, r0_ = 2 * g + hh, hh * D
                                    nc.vector.tensor_scalar_add(
                                        kTp[r0_:r0_ + D,
                                            h_ * N + n0: h_ * N + n0 + 512],
                                        ps[r0_:r0_ + D, :],
                                        b_col[r0_:r0_ + D, g:g + 1])
                # v
                for nq in range(4):
                    for ml in range(NQ // 128):
                        mc = nq * (NQ // 128) + ml
                        ps = vpp.tile([128, 512], F32, tag="v",
                                      name=f"v{nq}_{ml}")
                        for c in range(8):
                            nc.tensor.matmul(
                                ps,
                                x_slice(nq, c, ml * 128, (ml + 1) * 128),
                                wv_sb[:, c * 512:(c + 1) * 512],
                                start=(c == 0),
                                stop=(c == 7 and not with_biases))
                        if with_biases:
                            nc.tensor.matmul(ps, ones[0:1, 0:128],
                                             bv_sb[0:1, :],
                                             start=False, stop=True)
                        dst = v_sb[:, mc * HL * VS:(mc + 1) * HL * VS].rearrange(
                            "p (h e) -> p h e", e=VS)[:, :, 0:D]
                        nc.vector.tensor_copy(
                            dst, ps.rearrange("p (h e) -> p h e", e=D))

            # attn_outT (bf16, 16KB/partition)
            aoT = bigp.tile([128, 4 * N], MDT, tag="big", name="aoT")

            # ---------- Phase B + C: flat attention stream + proj ----------
            # Flat stream over (head-unit u = nh*8+h, m-chunk mcc): per chunk
            # emit [av(u, mcc - lag), sc(u, mcc), exp(u, mcc)].  The av lag
            # staircase (lag 3 for mcc 0/1, else 2) gives the previous head's
            # avs evacuation 2 chunks of slack before its PSUM slot is
            # reused, so the in-order PE queue never stalls on the DVE.
            # Softmax normalization: denominator row (av row 64) -> [128,8]
            # via sbuf-to-sbuf DMA -> DVE reciprocal -> partition-broadcast
            # DMA to [64,1024] -> one DVE multiply into aoT.  No PE involved.
            # Proj units (one output n-chunk nch x 512 cols) are injected at
            # head boundaries once their source half's aoT is complete.
            with tc.tile_pool(name="wppool", bufs=1) as wppool, \
                 tc.tile_pool(name="expp", bufs=8) as expp, \
                 tc.tile_pool(name="avsp", bufs=4) as avsp, \
                 tc.tile_pool(name="denp", bufs=4) as denp, \
                 tc.tile_pool(name="bcp", bufs=3) as bcp, \
                 tc.tile_pool(name="bpp", bufs=1) as bpp, \
                 tc.tile_pool(name="pout", bufs=3) as pout:
                wp_sb = wppool.tile([128, 4 * C], MDT, tag="wp", name="wp_sb")
                nc.sync.dma_start(
                    out=wp_sb.rearrange("p (g c) -> p g c", c=C),
                    in_=wp.rearrange("(g p) c -> p g c", p=128))
                bp_sb = bpp.tile([1, C], MDT, tag="bp")
                nc.sync.dma_start(out=bp_sb, in_=bp)

                with tc.tile_pool(name="scp", bufs=2, space="PSUM") as scp, \
                     tc.tile_pool(name="avp", bufs=1, space="PSUM") as avp, \
                     tc.tile_pool(name="pjp", bufs=2, space="PSUM") as pjp:

                    tails = []          # deferred normalization multiplies
                    proj_pending = []   # proj closures for a finished half
                    po_cur = {}         # nch -> pout tile awaiting 2nd jg

                    def emit_proj(nch, jg):
                        ps = pjp.tile([128, 512], F32, tag="pj",
                                      name=f"pj{nch}_{jg}")
                        for g in range(4):
                            nc.tensor.matmul(
                                ps,
                                aoT[:, g * N + nch * 128:
                                    g * N + (nch + 1) * 128],
                                wp_sb[:, g * C + jg * 512:
                                      g * C + jg * 512 + 512],
                                start=(g == 0),
                                stop=(g == 3 and not with_biases))
                        if with_biases:
                            nc.tensor.matmul(
                                ps, ones[0:1, 0:128],
                                bp_sb[0:1, jg * 512:(jg + 1) * 512],
                                start=False, stop=True)
                        if nch not in po_cur:
                            po_cur[nch] = pout.tile([128, C], F32, tag="po",
                                                    name=f"po{nch}")
                        po = po_cur[nch]
                        nc.vector.tensor_copy(
                            po[:, jg * 512:(jg + 1) * 512], ps)
                        if nch >= 14:
                            # final units: per-jg DMAs so the last transfer
                            # draining at kernel end is 256KB, not 512KB
                            nc.sync.dma_start(
                                out=out[nch * 128:(nch + 1) * 128,
                                        jg * 512:(jg + 1) * 512],
                                in_=po[:, jg * 512:(jg + 1) * 512])
                            if jg == 1:
                                del po_cur[nch]
                        elif jg == 1:
                            nc.sync.dma_start(
                                out=out[nch * 128:(nch + 1) * 128, :],
                                in_=po)
                            del po_cur[nch]

                    # per-head state
                    av_t = [None] * 16
                    ex_t = [[None] * 16 for _ in range(16)]

                    def emit_sc_exp(u, mcc):
                        nh, h = divmod(u, HL)
                        g, n0 = h // 2, nh * NHALF
                        sc = scp.tile([128, NHALF], F32, tag="sc",
                                      name=f"sc{u}_{mcc}")
                        for ngl in range(2):
                            nc.tensor.matmul(
                                sc[:, ngl * 512:(ngl + 1) * 512],
                                kTp[:, h * N + mcc * 128:
                                    h * N + (mcc + 1) * 128],
                                qT[:, g * N + n0 + ngl * 512:
                                   g * N + n0 + (ngl + 1) * 512],
                                start=True, stop=True)
                        ex = expp.tile([128, NHALF], BF16, tag="ex",
                                       name=f"ex{u}_{mcc}")
                        nc.scalar.activation(ex, sc, AFT.Exp, scale=SCALE)
                        ex_t[u][mcc] = ex

                    def emit_av(u, mcc):
                        h = u % HL
                        if mcc == 0:
                            av_t[u] = avp.tile([VS, NHALF], F32, tag="av",
                                               name=f"av{u}")
                        av = av_t[u]
                        ex = ex_t[u][mcc]
                        ex_t[u][mcc] = None
                        for ngl in range(2):
                            nc.tensor.matmul(
                                av[:, ngl * 512:(ngl + 1) * 512],
                                v_sb[:, (mcc * HL + h) * VS:
                                     (mcc * HL + h + 1) * VS],
                                ex[:, ngl * 512:(ngl + 1) * 512],
                                start=(mcc == 0), stop=(mcc == 15))

                    recips = []         # deferred reciprocal chains

                    def head_finish(u):
                        """avs evacuation for head u.  The reciprocal chain
                        is deferred a few chunks (so the DVE reciprocal's
                        den-DMA input has landed before it hits the in-order
                        DVE queue); the normalization multiply is deferred
                        three heads so its broadcast-DMA chain (~8us) never
                        delays the in-order DVE queue.  The very last head
                        uses a PE broadcast matmul instead of the broadcast
                        DMA to shorten the end-of-kernel serial chain."""
                        nh, h = divmod(u, HL)
                        g, r0, n0 = h // 2, (h % 2) * D, nh * NHALF
                        last = (u == 15)
                        av = av_t[u]
                        avs = avsp.tile([VS, NHALF], MDT, tag="avs",
                                        name=f"avs{u}")
                        nc.vector.tensor_copy(avs, av[0:VS, :])
                        den = denp.tile([128, NHALF // 128], MDT,
                                        tag="den", name=f"den{u}")
                        nc.gpsimd.dma_start(out=den, in_=avs[D:VS, :])
                        bcast = bcp.tile([D, NHALF], MDT, tag="bc",
                                         name=f"bc{u}")
                        rrow = denp.tile([1, NHALF], MDT, tag="rrow",
                                         name=f"rrow{u}")

                        def recip_chain():
                            rcp = denp.tile([128, NHALF // 128], MDT,
                                            tag="rcp", name=f"rcp{u}")
                            with nc.allow_low_precision(reason="softmax den"):
                                nc.vector.reciprocal(rcp, den)
                            nc.gpsimd.dma_start(out=rrow, in_=rcp)
                            if not last:
                                # replicate the reciprocal row across 64
                                # partitions (64 x 2KB descriptors) so the
                                # normalization is one DVE multiply -- no PE
                                # broadcast matmul needed
                                nc.gpsimd.dma_start(
                                    out=bcast,
                                    in_=rrow.unsqueeze(1).broadcast_to(
                                        [1, D, NHALF]))
                        recips.append(recip_chain)

                        def tail():
                            if last:
                                bc = scp.tile([D, NHALF], F32, tag="sc",
                                              name="bc15")
                                for ngl in range(2):
                                    nc.tensor.matmul(
                                        bc[:, ngl * 512:(ngl + 1) * 512],
                                        ones[0:1, 0:D],
                                        rrow[0:1, ngl * 512:(ngl + 1) * 512],
                                        start=True, stop=True)
                                src1 = bc
                            else:
                                src1 = bcast
                            nc.vector.tensor_mul(
                                aoT[r0:r0 + D,
                                    g * N + n0: g * N + n0 + NHALF],
                                avs[0:D, :], src1)
                        tails.append(tail)
                        if len(tails) > 2:
                            tails.pop(0)()

                    # av emission schedule per head chunk hc (0..15):
                    # doubles at hc 5/6, lag 3 afterwards, the last three
                    # m-chunks handled at the next head's hc 0..2.  The wide
                    # gap (hc2 boundary -> hc5 reuse) covers the previous
                    # head's avs-evacuation latency so the in-order PE queue
                    # never waits on the DVE.
                    def av_due(u, hc):
                        due = []
                        if u > 0 and hc <= 2:
                            due.append((u - 1, 13 + hc))
                        if hc == 5:
                            due.extend([(u, 0), (u, 1)])
                        elif hc == 6:
                            due.extend([(u, 2), (u, 3)])
                        elif hc >= 7:
                            due.append((u, hc - 3))
                        return due

                    for u in range(16):
                        for hc in range(16):
                            for (ua, mcc) in av_due(u, hc):
                                emit_av(ua, mcc)
                                if mcc == 15:
                                    head_finish(ua)
                                    # inject proj work for the finished half
                                    # (from u=11 so the last tail multiply of
                                    # that half is a full head ahead)
                                    if u >= 11:
                                        for _ in range(4):
                                            if proj_pending:
                                                proj_pending.pop(0)()
                            if hc == 6 and recips:
                                recips.pop(0)()
                            emit_sc_exp(u, hc)
                        if u == 7:
                            for nch in range(8):
                                for jg in range(2):
                                    proj_pending.append(
                                        lambda nch=nch, jg=jg:
                                            emit_proj(nch, jg))
                    # flush: remaining av chunks + last head's chain
                    emit_av(15, 13)
                    emit_av(15, 14)
                    emit_av(15, 15)
                    head_finish(15)
                    while recips:
                        recips.pop(0)()
                    while tails:
                        tails.pop(0)()
                    while proj_pending:
                        proj_pending.pop(0)()
                    for nch in range(8, 16):
                        for jg in range(2):
                            emit_proj(nch, jg)
    return nc


def _bf16(a):
    import ml_dtypes
    return np.ascontiguousarray(a).astype(ml_dtypes.bfloat16)


def shard_inputs(x, Wqkv, bqkv, Wproj, bproj):
    """Full inputs -> per-core in_maps. Core c: batch c//2, head-group c%2."""
    in_maps = []
    for core in range(N_CORES):
        b, hg = core // 2, core % 2
        s = hg * 512
        m = {
            "xt": _bf16(x[b].T),
            "wq": _bf16(Wqkv[:, s:s + 512]),
            "wk": _bf16(Wqkv[:, C + s: C + s + 512]),
            "wv": _bf16(Wqkv[:, 2 * C + s: 2 * C + s + 512]),
            "wp": _bf16(Wproj[s:s + 512, :]),
            "bqc": np.ascontiguousarray(bqkv[s:s + 512].reshape(4, 128).T),
            "bkc": np.ascontiguousarray(bqkv[C + s: C + s + 512].reshape(4, 128).T),
            "bv": _bf16(bqkv[2 * C + s: 2 * C + s + 512][None, :]),
            "bp": _bf16(
                (bproj if hg == 0 else np.zeros_like(bproj))[None, :]),
            "ones_row": _bf16(np.ones((1, 512), np.float32)),
        }
        in_maps.append(m)
    return in_maps


def unshard_output(results):
    """Per-core partial outputs -> full [4, N, C]."""
    outs = []
    for b in range(4):
        outs.append(results[2 * b]["out"] + results[2 * b + 1]["out"])
    return np.stack(outs, axis=0)


# revision 17
# speedup vs baseline: 1.0882x; 1.0190x over previous
"""Self-contained Trainium2 Bass kernel: 16-head self-attention (B=4, N=2048,
C=1024, fp32), SPMD across 8 NeuronCores.

Entry point: kernel(**inputs) -> np.ndarray matching the reference module
(qkv projection + scaled-dot-product softmax attention + output projection).
See build_nc() docstring for the kernel design.
"""
import numpy as np

_NC_CACHE = {}


def kernel(x, Wqkv, bqkv, Wproj, bproj):
    from concourse.bass_utils import run_bass_kernel_spmd
    x = np.asarray(x, dtype=np.float32)
    Wqkv = np.asarray(Wqkv, dtype=np.float32)
    bqkv = np.asarray(bqkv, dtype=np.float32)
    Wproj = np.asarray(Wproj, dtype=np.float32)
    bproj = np.asarray(bproj, dtype=np.float32)
    # the rank-1 bias accumulation steps are emitted only when any bias is
    # actually nonzero (they are exact zeros in this problem's inputs)
    wb = bool(np.any(bqkv) or np.any(bproj))
    if wb not in _NC_CACHE:
        nc = build_nc(with_biases=wb)
        split_excess_waits(nc)
        _NC_CACHE[wb] = nc
    nc = _NC_CACHE[wb]
    in_maps = shard_inputs(x, Wqkv, bqkv, Wproj, bproj)
    res = run_bass_kernel_spmd(nc, in_maps, core_ids=list(range(N_CORES)))
    return unshard_output(res.results).astype(np.float32)


# ======================================================================
# IR post-pass: this walrus build accepts at most one semaphore wait per
# instruction; overflow waits move onto chained NoOps just before the
# instruction on the same engine queue.
# ======================================================================

# Walrus TPB_CTRL codegen (Drain/NoOp lowering) accepts only 1 sync wait;
# regular engine instructions accept more (tested empirically).
CTRL_OPCODES = {"Drain", "NoOp", "EventSemaphore", "AllEngineBarrier"}

def split_excess_waits(nc, engine_max=1, ctrl_max=1):
    n_split = 0
    for f in nc.m.functions:
        for bb in f.blocks:
            insts = list(bb.instructions)
            out = []
            changed = False
            for inst in insts:
                si = inst.sync_info
                max_w = ctrl_max if inst.opcode in CTRL_OPCODES else engine_max
                if si is not None and si.on_wait and len(si.on_wait) > max_w:
                    waits = list(si.on_wait)
                    extra, keep = waits[max_w:], waits[:max_w]
                    for i in range(0, len(extra), ctrl_max):
                        nop = bass_rust.InstNoOp(
                            name=f"{inst.name}-wsplit{i}", ins=[], outs=[])
                        nop.engine = inst.engine
                        nop.sync_info = mybir.SyncInfo(
                            on_wait=extra[i:i + ctrl_max], on_update=[])
                        out.append(nop)
                        n_split += 1
                    inst.sync_info = mybir.SyncInfo(
                        on_wait=keep, on_update=list(si.on_update))
                    changed = True
                out.append(inst)
            if changed:
                bb.instructions = out
    return n_split


# ======================================================================
# Kernel proper
# ======================================================================
import bass_rust
import concourse.bass as bass
import concourse.tile as tile
import concourse.mybir as mybir


F32 = mybir.dt.float32
BF16 = mybir.dt.bfloat16

N = 2048        # sequence length
C = 1024        # embed dim
HL = 8          # heads handled per core
D = 64          # head dim
SCALE = D ** -0.5
NHALF = N // 2
NQ = N // 4     # x tile quarter
VS = D + 1      # v columns per head incl. ones column
N_CORES = 8

AFT = mybir.ActivationFunctionType
ALU = mybir.AluOpType


def build_nc(with_biases=True):
    MDT = BF16
    nc = bass.Bass("TRN2", target_bir_lowering=False, debug=False,
                   num_devices=N_CORES)
    xt = nc.dram_tensor("xt", [C, N], MDT, kind="ExternalInput").ap()
    wq = nc.dram_tensor("wq", [C, HL * D], MDT, kind="ExternalInput").ap()
    wk = nc.dram_tensor("wk", [C, HL * D], MDT, kind="ExternalInput").ap()
    wv = nc.dram_tensor("wv", [C, HL * D], MDT, kind="ExternalInput").ap()
    wp = nc.dram_tensor("wp", [HL * D, C], MDT, kind="ExternalInput").ap()
    bqc = nc.dram_tensor("bqc", [128, 4], F32, kind="ExternalInput").ap()
    bkc = nc.dram_tensor("bkc", [128, 4], F32, kind="ExternalInput").ap()
    bv = nc.dram_tensor("bv", [1, HL * D], MDT, kind="ExternalInput").ap()
    bp = nc.dram_tensor("bp", [1, C], MDT, kind="ExternalInput").ap()
    ones_row = nc.dram_tensor("ones_row", [1, 512], MDT, kind="ExternalInput").ap()
    out = nc.dram_tensor("out", [N, C], F32, kind="ExternalOutput").ap()

    with tile.TileContext(nc) as tc:
        with tc.tile_pool(name="consts", bufs=1) as consts, \
             tc.tile_pool(name="persist", bufs=1) as persist, \
             tc.tile_pool(name="big", bufs=1) as bigp:

            # persistent activation tiles (bf16 so the attention matmuls
            # get full 128x128 stationary tiles + fast weight loads):
            #   qT: pair-packed [2 heads' d x 128, pair-group x n]
            #   kTp: per-head [128, head x n] with the partner head's 64
            #        partition rows zeroed (K=128 contraction, zeros kill
            #        the partner-q contribution in the shared qT rhs)
            #   v_sb: per (m-chunk, head) [128, 65]: cols 0:64 v, col 64
            #        ones (softmax denominator)
            qT = persist.tile([128, 4 * N], BF16, tag="qT")
            kTp = persist.tile([128, HL * N], BF16, tag="kTp")
            v_sb = persist.tile([128, 16 * HL * VS], BF16, tag="v")
            nc.gpsimd.memset(kTp, 0.0)
            vview = v_sb.rearrange("p (m h e) -> p m h e", h=HL, e=VS)
            nc.gpsimd.memset(vview[:, :, :, D:D + 1], 1.0)

            # ---------- Phase A: xT, qT/kT, v ----------
            # One consolidated dma_start per tensor; x in quarter tiles so
            # the first matmul group waits on 1MB, not 4MB.
            with tc.tile_pool(name="wpool", bufs=3) as wpool, \
                 tc.tile_pool(name="biases", bufs=1) as biasp, \
                 tc.tile_pool(name="xpool", bufs=4) as xpool, \
                 tc.tile_pool(name="kqp", bufs=2, space="PSUM") as kqp, \
                 tc.tile_pool(name="vpp", bufs=2, space="PSUM") as vpp:
                # wq and x quarter 0 are split into half-tiles (c-chunks 0-3
                # / 4-7) so the very first matmuls wait on 1MB, not 2MB
                wq_sbh = [wpool.tile([128, 4 * 512], MDT, tag="wh",
                                     name=f"wq_sb{i}") for i in range(2)]
                wk_sb = wpool.tile([128, 8 * 512], MDT, tag="w", name="wk_sb")
                wv_sb = wpool.tile([128, 8 * 512], MDT, tag="w", name="wv_sb")
                xT0h = [xpool.tile([128, 4 * NQ], MDT, tag="xh",
                                   name=f"xT0{i}") for i in range(2)]
                xTq = [None] + [xpool.tile([128, 8 * NQ], MDT, tag="x",
                                           name=f"xT{i}") for i in range(1, 4)]

                def wq_slice(c, g):
                    return wq_sbh[c // 4][:, (c % 4) * 512 + g * 128:
                                          (c % 4) * 512 + (g + 1) * 128]

                def wk_slice(c, g):
                    return wk_sb[:, c * 512 + g * 128:c * 512 + (g + 1) * 128]

                def x_slice(nq, c, lo, hi):
                    if nq == 0:
                        return xT0h[c // 4][:, (c % 4) * NQ + lo:
                                            (c % 4) * NQ + hi]
                    return xTq[nq][:, c * NQ + lo:c * NQ + hi]

                def load_w(dst, src):
                    nc.sync.dma_start(
                        out=dst.rearrange("p (c d) -> p c d", d=512),
                        in_=src.rearrange("(c p) d -> p c d", p=128))

                load_w(wq_sbh[0], wq[0:512, :])
                nc.sync.dma_start(
                    out=xT0h[0].rearrange("p (c n) -> p c n", n=NQ),
                    in_=xt[0:512, 0:NQ].rearrange("(c p) n -> p c n", p=128))
                load_w(wq_sbh[1], wq[512:1024, :])
                nc.sync.dma_start(
                    out=xT0h[1].rearrange("p (c n) -> p c n", n=NQ),
                    in_=xt[512:1024, 0:NQ].rearrange("(c p) n -> p c n", p=128))
                ones = consts.tile([1, 512], MDT, tag="ones")
                nc.sync.dma_start(out=ones, in_=ones_row)
                bqc_sb = biasp.tile([128, 4], F32, tag="bqc")
                bkc_sb = biasp.tile([128, 4], F32, tag="bkc")
                bv_sb = biasp.tile([1, HL * D], MDT, tag="bv")
                nc.sync.dma_start(out=bqc_sb, in_=bqc)
                nc.sync.dma_start(out=bkc_sb, in_=bkc)
                nc.sync.dma_start(out=bv_sb, in_=bv)
                nc.sync.dma_start(
                    out=xTq[1].rearrange("p (c n) -> p c n", n=NQ),
                    in_=xt[:, NQ:2 * NQ].rearrange("(c p) n -> p c n", p=128))
                load_w(wk_sb, wk)
                nc.sync.dma_start(
                    out=xTq[2].rearrange("p (c n) -> p c n", n=NQ),
                    in_=xt[:, 2 * NQ:3 * NQ].rearrange("(c p) n -> p c n", p=128))
                nc.sync.dma_start(
                    out=xTq[3].rearrange("p (c n) -> p c n", n=NQ),
                    in_=xt[:, 3 * NQ:4 * NQ].rearrange("(c p) n -> p c n", p=128))
                load_w(wv_sb, wv)

                # qT / kT (bias folded into evacuation)
                for dstT, w_slice, b_col in ((qT, wq_slice, bqc_sb),
                                             (kTp, wk_slice, bkc_sb)):
                    for nq in range(4):
                        for g in range(4):
                            ps = kqp.tile([128, 512], F32, tag="kq",
                                          name=f"kq{nq}_{g}")
                            for c in range(8):
                                nc.tensor.matmul(
                                    ps,
                                    w_slice(c, g),
                                    x_slice(nq, c, 0, 512),
                                    start=(c == 0), stop=(c == 7))
                            n0 = nq * NQ
                            if dstT is qT:
                                nc.vector.tensor_scalar_add(
                                    qT[:, g * N + n0: g * N + n0 + 512],
                                    ps, b_col[:, g:g + 1])
                            else:
                                for hh in range(2):
                                    h_, r0_ = 2 * g + hh, hh * D
                                    nc.vector.tensor_scalar_add(
                                        kTp[r0_:r0_ + D,
                                            h_ * N + n0: h_ * N + n0 + 512],
                                        ps[r0_:r0_ + D, :],
                                        b_col[r0_:r0_ + D, g:g + 1])
                # v
                for nq in range(4):
                    for ml in range(NQ // 128):
                        mc = nq * (NQ // 128) + ml
                        ps = vpp.tile([128, 512], F32, tag="v",
                                      name=f"v{nq}_{ml}")
                        for c in range(8):
                            nc.tensor.matmul(
                                ps,
                                x_slice(nq, c, ml * 128, (ml + 1) * 128),
                                wv_sb[:, c * 512:(c + 1) * 512],
                                start=(c == 0),
                                stop=(c == 7 and not with_biases))
                        if with_biases:
                            nc.tensor.matmul(ps, ones[0:1, 0:128],
                                             bv_sb[0:1, :],
                                             start=False, stop=True)
                        dst = v_sb[:, mc * HL * VS:(mc + 1) * HL * VS].rearrange(
                            "p (h e) -> p h e", e=VS)[:, :, 0:D]
                        nc.vector.tensor_copy(
                            dst, ps.rearrange("p (h e) -> p h e", e=D))

            # attn_outT (bf16, 16KB/partition)
            aoT = bigp.tile([128, 4 * N], MDT, tag="big", name="aoT")

            # ---------- Phase B + C: flat attention stream + proj ----------
            # Flat stream over (head-unit u = nh*8+h, m-chunk mcc): per chunk
            # emit [av(u, mcc - lag), sc(u, mcc), exp(u, mcc)].  The av lag
            # staircase (lag 3 for mcc 0/1, else 2) gives the previous head's
            # avs evacuation 2 chunks of slack before its PSUM slot is
            # reused, so the in-order PE queue never stalls on the DVE.
            # Softmax normalization: denominator row (av row 64) -> [128,8]
            # via sbuf-to-sbuf DMA -> DVE reciprocal -> partition-broadcast
            # DMA to [64,1024] -> one DVE multiply into aoT.  No PE involved.
            # Proj units (one output n-chunk nch x 512 cols) are injected at
            # head boundaries once their source half's aoT is complete.
            with tc.tile_pool(name="wppool", bufs=1) as wppool, \
                 tc.tile_pool(name="expp", bufs=8) as expp, \
                 tc.tile_pool(name="avsp", bufs=5) as avsp, \
                 tc.tile_pool(name="denp", bufs=4) as denp, \
                 tc.tile_pool(name="bcp", bufs=4) as bcp, \
                 tc.tile_pool(name="bpp", bufs=1) as bpp, \
                 tc.tile_pool(name="pout", bufs=3) as pout:
                wp_sb = wppool.tile([128, 4 * C], MDT, tag="wp", name="wp_sb")
                nc.sync.dma_start(
                    out=wp_sb.rearrange("p (g c) -> p g c", c=C),
                    in_=wp.rearrange("(g p) c -> p g c", p=128))
                bp_sb = bpp.tile([1, C], MDT, tag="bp")
                nc.sync.dma_start(out=bp_sb, in_=bp)

                with tc.tile_pool(name="scp", bufs=2, space="PSUM") as scp, \
                     tc.tile_pool(name="avp", bufs=1, space="PSUM") as avp, \
                     tc.tile_pool(name="pjp", bufs=2, space="PSUM") as pjp:

                    tails = []          # deferred normalization multiplies
                    proj_pending = []   # proj closures for a finished half
                    po_cur = {}         # nch -> pout tile awaiting 2nd jg

                    def emit_proj(nch, jg):
                        ps = pjp.tile([128, 512], F32, tag="pj",
                                      name=f"pj{nch}_{jg}")
                        for g in range(4):
                            nc.tensor.matmul(
                                ps,
                                aoT[:, g * N + nch * 128:
                                    g * N + (nch + 1) * 128],
                                wp_sb[:, g * C + jg * 512:
                                      g * C + jg * 512 + 512],
                                start=(g == 0),
                                stop=(g == 3 and not with_biases))
                        if with_biases:
                            nc.tensor.matmul(
                                ps, ones[0:1, 0:128],
                                bp_sb[0:1, jg * 512:(jg + 1) * 512],
                                start=False, stop=True)
                        if nch not in po_cur:
                            po_cur[nch] = pout.tile([128, C], F32, tag="po",
                                                    name=f"po{nch}")
                        po = po_cur[nch]
                        nc.vector.tensor_copy(
                            po[:, jg * 512:(jg + 1) * 512], ps)
                        if nch >= 14:
                            # final units: per-jg DMAs so the last transfer
                            # draining at kernel end is 256KB, not 512KB
                            nc.sync.dma_start(
                                out=out[nch * 128:(nch + 1) * 128,
                                        jg * 512:(jg + 1) * 512],
                                in_=po[:, jg * 512:(jg + 1) * 512])
                            if jg == 1:
                                del po_cur[nch]
                        elif jg == 1:
                            nc.sync.dma_start(
                                out=out[nch * 128:(nch + 1) * 128, :],
                                in_=po)
                            del po_cur[nch]

                    # per-head state
                    av_t = [None] * 16
                    ex_t = [[None] * 16 for _ in range(16)]

                    def emit_sc_exp(u, mcc):
                        nh, h = divmod(u, HL)
                        g, n0 = h // 2, nh * NHALF
                        sc = scp.tile([128, NHALF], F32, tag="sc",
                                      name=f"sc{u}_{mcc}")
                        for ngl in range(2):
                            nc.tensor.matmul(
                                sc[:, ngl * 512:(ngl + 1) * 512],
                                kTp[:, h * N + mcc * 128:
                                    h * N + (mcc + 1) * 128],
                                qT[:, g * N + n0 + ngl * 512:
                                   g * N + n0 + (ngl + 1) * 512],
                                start=True, stop=True)
                        ex = expp.tile([128, NHALF], BF16, tag="ex",
                                       name=f"ex{u}_{mcc}")
                        nc.scalar.activation(ex, sc, AFT.Exp, scale=SCALE)
                        ex_t[u][mcc] = ex

                    def emit_av(u, mcc):
                        h = u % HL
                        if mcc == 0:
                            av_t[u] = avp.tile([VS, NHALF], F32, tag="av",
                                               name=f"av{u}")
                        av = av_t[u]
                        ex = ex_t[u][mcc]
                        ex_t[u][mcc] = None
                        for ngl in range(2):
                            nc.tensor.matmul(
                                av[:, ngl * 512:(ngl + 1) * 512],
                                v_sb[:, (mcc * HL + h) * VS:
                                     (mcc * HL + h + 1) * VS],
                                ex[:, ngl * 512:(ngl + 1) * 512],
                                start=(mcc == 0), stop=(mcc == 15))

                    recips = []         # deferred reciprocal chains

                    def head_finish(u):
                        """avs evacuation for head u.  The reciprocal chain
                        is deferred a few chunks (so the DVE reciprocal's
                        den-DMA input has landed before it hits the in-order
                        DVE queue); the normalization multiply is deferred
                        three heads so its broadcast-DMA chain (~8us) never
                        delays the in-order DVE queue.  The very last head
                        uses a PE broadcast matmul instead of the broadcast
                        DMA to shorten the end-of-kernel serial chain."""
                        nh, h = divmod(u, HL)
                        g, r0, n0 = h // 2, (h % 2) * D, nh * NHALF
                        last = (u == 15)
                        av = av_t[u]
                        avs = avsp.tile([VS, NHALF], MDT, tag="avs",
                                        name=f"avs{u}")
                        nc.vector.tensor_copy(avs, av[0:VS, :])
                        den = denp.tile([128, NHALF // 128], MDT,
                                        tag="den", name=f"den{u}")
                        nc.sync.dma_start(out=den, in_=avs[D:VS, :])
                        bcast = bcp.tile([D, NHALF], MDT, tag="bc",
                                         name=f"bc{u}")
                        rrow = denp.tile([1, NHALF], MDT, tag="rrow",
                                         name=f"rrow{u}")

                        def recip_chain():
                            rcp = denp.tile([128, NHALF // 128], MDT,
                                            tag="rcp", name=f"rcp{u}")
                            with nc.allow_low_precision(reason="softmax den"):
                                nc.vector.reciprocal(rcp, den)
                            nc.gpsimd.dma_start(out=rrow, in_=rcp)
                            if not last:
                                # replicate the reciprocal row across 64
                                # partitions (64 x 2KB descriptors) so the
                                # normalization is one DVE multiply -- no PE
                                # broadcast matmul needed
                                nc.gpsimd.dma_start(
                                    out=bcast,
                                    in_=rrow.unsqueeze(1).broadcast_to(
                                        [1, D, NHALF]))
                        recips.append(recip_chain)

                        def tail():
                            if last:
                                bc = scp.tile([D, NHALF], F32, tag="sc",
                                              name="bc15")
                                for ngl in range(2):
                                    nc.tensor.matmul(
                                        bc[:, ngl * 512:(ngl + 1) * 512],
                                        ones[0:1, 0:D],
                                        rrow[0:1, ngl * 512:(ngl + 1) * 512],
                                        start=True, stop=True)
                                src1 = bc
                            else:
                                src1 = bcast
                            nc.vector.tensor_mul(
                                aoT[r0:r0 + D,
                                    g * N + n0: g * N + n0 + NHALF],
                                avs[0:D, :], src1)
                        tails.append(tail)
                        if len(tails) > 3:
                            tails.pop(0)()

                    # av emission schedule per head chunk hc (0..15):
                    # doubles at hc 5/6, lag 3 afterwards, the last three
                    # m-chunks handled at the next head's hc 0..2.  The wide
                    # gap (hc2 boundary -> hc5 reuse) covers the previous
                    # head's avs-evacuation latency so the in-order PE queue
                    # never waits on the DVE.
                    def av_due(u, hc):
                        due = []
                        if u > 0 and hc <= 2:
                            due.append((u - 1, 13 + hc))
                        if hc == 5:
                            due.extend([(u, 0), (u, 1)])
                        elif hc == 6:
                            due.extend([(u, 2), (u, 3)])
                        elif hc >= 7:
                            due.append((u, hc - 3))
                        return due

                    for u in range(16):
                        for hc in range(16):
                            for (ua, mcc) in av_due(u, hc):
                                emit_av(ua, mcc)
                                if mcc == 15:
                                    head_finish(ua)
                                    # inject proj work for the finished half
                                    # (from u=12 so the last tail multiply of
                                    # that half is a full head ahead)
                                    if u >= 12:
                                        for _ in range(4):
                                            if proj_pending:
                                                proj_pending.pop(0)()
                            if hc == 6 and recips:
                                recips.pop(0)()
                            emit_sc_exp(u, hc)
                        if u == 7:
                            for nch in range(8):
                                for jg in range(2):
                                    proj_pending.append(
                                        lambda nch=nch, jg=jg:
                                            emit_proj(nch, jg))
                    # flush: remaining av chunks + last head's chain
                    emit_av(15, 13)
                    emit_av(15, 14)
                    emit_av(15, 15)
                    head_finish(15)
                    while recips:
                        recips.pop(0)()
                    while tails:
                        tails.pop(0)()
                    while proj_pending:
                        proj_pending.pop(0)()
                    for nch in range(8, 16):
                        for jg in range(2):
                            emit_proj(nch, jg)
    return nc


def _bf16(a):
    import ml_dtypes
    return np.ascontiguousarray(a).astype(ml_dtypes.bfloat16)


def shard_inputs(x, Wqkv, bqkv, Wproj, bproj):
    """Full inputs -> per-core in_maps. Core c: batch c//2, head-group c%2."""
    in_maps = []
    for core in range(N_CORES):
        b, hg = core // 2, core % 2
        s = hg * 512
        m = {
            "xt": _bf16(x[b].T),
            "wq": _bf16(Wqkv[:, s:s + 512]),
            "wk": _bf16(Wqkv[:, C + s: C + s + 512]),
            "wv": _bf16(Wqkv[:, 2 * C + s: 2 * C + s + 512]),
            "wp": _bf16(Wproj[s:s + 512, :]),
            "bqc": np.ascontiguousarray(bqkv[s:s + 512].reshape(4, 128).T),
            "bkc": np.ascontiguousarray(bqkv[C + s: C + s + 512].reshape(4, 128).T),
            "bv": _bf16(bqkv[2 * C + s: 2 * C + s + 512][None, :]),
            "bp": _bf16(
                (bproj if hg == 0 else np.zeros_like(bproj))[None, :]),
            "ones_row": _bf16(np.ones((1, 512), np.float32)),
        }
        in_maps.append(m)
    return in_maps


def unshard_output(results):
    """Per-core partial outputs -> full [4, N, C]."""
    outs = []
    for b in range(4):
        outs.append(results[2 * b]["out"] + results[2 * b + 1]["out"])
    return np.stack(outs, axis=0)


# revision 19
# speedup vs baseline: 1.1048x; 1.0152x over previous
"""Self-contained Trainium2 Bass kernel: 16-head self-attention (B=4, N=2048,
C=1024, fp32), SPMD across 8 NeuronCores.

Entry point: kernel(**inputs) -> np.ndarray matching the reference module
(qkv projection + scaled-dot-product softmax attention + output projection).
See build_nc() docstring for the kernel design.
"""
import numpy as np

_NC_CACHE = {}


def kernel(x, Wqkv, bqkv, Wproj, bproj):
    from concourse.bass_utils import run_bass_kernel_spmd
    x = np.asarray(x, dtype=np.float32)
    Wqkv = np.asarray(Wqkv, dtype=np.float32)
    bqkv = np.asarray(bqkv, dtype=np.float32)
    Wproj = np.asarray(Wproj, dtype=np.float32)
    bproj = np.asarray(bproj, dtype=np.float32)
    # the rank-1 bias accumulation steps are emitted only when any bias is
    # actually nonzero (they are exact zeros in this problem's inputs)
    wb = bool(np.any(bqkv) or np.any(bproj))
    if wb not in _NC_CACHE:
        nc = build_nc(with_biases=wb)
        split_excess_waits(nc)
        _NC_CACHE[wb] = nc
    nc = _NC_CACHE[wb]
    in_maps = shard_inputs(x, Wqkv, bqkv, Wproj, bproj)
    res = run_bass_kernel_spmd(nc, in_maps, core_ids=list(range(N_CORES)))
    return unshard_output(res.results).astype(np.float32)


# ======================================================================
# IR post-pass: this walrus build accepts at most one semaphore wait per
# instruction; overflow waits move onto chained NoOps just before the
# instruction on the same engine queue.
# ======================================================================

# Walrus TPB_CTRL codegen (Drain/NoOp lowering) accepts only 1 sync wait;
# regular engine instructions accept more (tested empirically).
CTRL_OPCODES = {"Drain", "NoOp", "EventSemaphore", "AllEngineBarrier"}

def split_excess_waits(nc, engine_max=1, ctrl_max=1):
    n_split = 0
    for f in nc.m.functions:
        for bb in f.blocks:
            insts = list(bb.instructions)
            out = []
            changed = False
            for inst in insts:
                si = inst.sync_info
                max_w = ctrl_max if inst.opcode in CTRL_OPCODES else engine_max
                if si is not None and si.on_wait and len(si.on_wait) > max_w:
                    waits = list(si.on_wait)
                    extra, keep = waits[max_w:], waits[:max_w]
                    for i in range(0, len(extra), ctrl_max):
                        nop = bass_rust.InstNoOp(
                            name=f"{inst.name}-wsplit{i}", ins=[], outs=[])
                        nop.engine = inst.engine
                        nop.sync_info = mybir.SyncInfo(
                            on_wait=extra[i:i + ctrl_max], on_update=[])
                        out.append(nop)
                        n_split += 1
                    inst.sync_info = mybir.SyncInfo(
                        on_wait=keep, on_update=list(si.on_update))
                    changed = True
                out.append(inst)
            if changed:
                bb.instructions = out
    return n_split


# ======================================================================
# Kernel proper
# ======================================================================
import bass_rust
import concourse.bass as bass
import concourse.tile as tile
import concourse.mybir as mybir


F32 = mybir.dt.float32
BF16 = mybir.dt.bfloat16

N = 2048        # sequence length
C = 1024        # embed dim
HL = 8          # heads handled per core
D = 64          # head dim
SCALE = D ** -0.5
NHALF = N // 2
NQ = N // 4     # x tile quarter
VS = D + 1      # v columns per head incl. ones column
N_CORES = 8

AFT = mybir.ActivationFunctionType
ALU = mybir.AluOpType


def build_nc(with_biases=True):
    MDT = BF16
    nc = bass.Bass("TRN2", target_bir_lowering=False, debug=False,
                   num_devices=N_CORES)
    xt = nc.dram_tensor("xt", [C, N], MDT, kind="ExternalInput").ap()
    wq = nc.dram_tensor("wq", [C, HL * D], MDT, kind="ExternalInput").ap()
    wk = nc.dram_tensor("wk", [C, HL * D], MDT, kind="ExternalInput").ap()
    wv = nc.dram_tensor("wv", [C, HL * D], MDT, kind="ExternalInput").ap()
    wp = nc.dram_tensor("wp", [HL * D, C], MDT, kind="ExternalInput").ap()
    bqc = nc.dram_tensor("bqc", [128, 4], F32, kind="ExternalInput").ap()
    bkc = nc.dram_tensor("bkc", [128, 4], F32, kind="ExternalInput").ap()
    bv = nc.dram_tensor("bv", [1, HL * D], MDT, kind="ExternalInput").ap()
    bp = nc.dram_tensor("bp", [1, C], MDT, kind="ExternalInput").ap()
    ones_row = nc.dram_tensor("ones_row", [1, 512], MDT, kind="ExternalInput").ap()
    out = nc.dram_tensor("out", [N, C], F32, kind="ExternalOutput").ap()

    with tile.TileContext(nc) as tc:
        with tc.tile_pool(name="consts", bufs=1) as consts, \
             tc.tile_pool(name="persist", bufs=1) as persist, \
             tc.tile_pool(name="big", bufs=1) as bigp, \
             tc.tile_pool(name="biases", bufs=1) as biasp, \
             tc.tile_pool(name="wqpool", bufs=2) as wqpool, \
             tc.tile_pool(name="xpool", bufs=4) as xpool:

            # persistent activation tiles (bf16 so the attention matmuls
            # get full 128x128 stationary tiles + fast weight loads):
            #   qT: pair-packed [2 heads' d x 128, pair-group x n]
            #   kTp: per-head [128, head x n] with the partner head's 64
            #        partition rows zeroed (K=128 contraction, zeros kill
            #        the partner-q contribution in the shared qT rhs)
            #   v_sb: per (m-chunk, head) [128, 65]: cols 0:64 v, col 64
            #        ones (softmax denominator)
            qT = persist.tile([128, 4 * N], BF16, tag="qT")
            kTp = persist.tile([128, HL * N], BF16, tag="kTp")
            v_sb = persist.tile([128, 16 * HL * VS], BF16, tag="v")
            nc.gpsimd.memset(kTp, 0.0)
            vview = v_sb.rearrange("p (m h e) -> p m h e", h=HL, e=VS)
            nc.gpsimd.memset(vview[:, :, :, D:D + 1], 1.0)
            aoT = bigp.tile([128, 4 * N], MDT, tag="big", name="aoT")

            # x quarters (quarter 0 as c-chunk half tiles so the first
            # matmuls wait on 0.5MB), wq halves, biases: these live through
            # phase B because the q projection for head-groups 1..3 runs
            # interleaved with the attention stream.
            xT0h = [xpool.tile([128, 4 * NQ], MDT, tag="xh",
                               name=f"xT0{i}") for i in range(2)]
            xTq = [None] + [xpool.tile([128, 8 * NQ], MDT, tag="x",
                                       name=f"xT{i}") for i in range(1, 4)]
            wq_sbh = [wqpool.tile([128, 4 * 512], MDT, tag="wh",
                                  name=f"wq_sb{i}") for i in range(2)]
            bqc_sb = biasp.tile([128, 4], F32, tag="bqc")
            bkc_sb = biasp.tile([128, 4], F32, tag="bkc")
            bv_sb = biasp.tile([1, HL * D], MDT, tag="bv")
            ones = consts.tile([1, 512], MDT, tag="ones")

            def wq_slice(c, g):
                return wq_sbh[c // 4][:, (c % 4) * 512 + g * 128:
                                      (c % 4) * 512 + (g + 1) * 128]

            def x_slice(nq, c, lo, hi):
                if nq == 0:
                    return xT0h[c // 4][:, (c % 4) * NQ + lo:
                                        (c % 4) * NQ + hi]
                return xTq[nq][:, c * NQ + lo:c * NQ + hi]

            def emit_q_group(nq, g, pool, tag):
                """One [128, 512] q output tile: contraction over C."""
                ps = pool.tile([128, 512], F32, tag=tag, name=f"q{nq}_{g}")
                for c in range(8):
                    nc.tensor.matmul(
                        ps, wq_slice(c, g), x_slice(nq, c, 0, 512),
                        start=(c == 0), stop=(c == 7))
                n0 = nq * NQ
                nc.vector.tensor_scalar_add(
                    qT[:, g * N + n0: g * N + n0 + 512],
                    ps, bqc_sb[:, g:g + 1])

            # ---------- Phase A: k and v for all m, then q group 0 ----------
            # (attention unit 0 only needs kTp, v_sb and qT group 0; the
            # remaining q groups are emitted inside the attention stream)
            with tc.tile_pool(name="wkv", bufs=2) as wkv, \
                 tc.tile_pool(name="kqp", bufs=2, space="PSUM") as kqp, \
                 tc.tile_pool(name="vpp", bufs=2, space="PSUM") as vpp:
                wk_sb = wkv.tile([128, 8 * 512], MDT, tag="w", name="wk_sb")
                wv_sb = wkv.tile([128, 8 * 512], MDT, tag="w", name="wv_sb")

                def load_w(dst, src):
                    nc.sync.dma_start(
                        out=dst.rearrange("p (c d) -> p c d", d=512),
                        in_=src.rearrange("(c p) d -> p c d", p=128))

                load_w(wk_sb, wk)
                nc.sync.dma_start(
                    out=xT0h[0].rearrange("p (c n) -> p c n", n=NQ),
                    in_=xt[0:512, 0:NQ].rearrange("(c p) n -> p c n", p=128))
                nc.sync.dma_start(
                    out=xT0h[1].rearrange("p (c n) -> p c n", n=NQ),
                    in_=xt[512:1024, 0:NQ].rearrange("(c p) n -> p c n", p=128))
                nc.sync.dma_start(out=ones, in_=ones_row)
                nc.sync.dma_start(out=bqc_sb, in_=bqc)
                nc.sync.dma_start(out=bkc_sb, in_=bkc)
                nc.sync.dma_start(out=bv_sb, in_=bv)
                nc.sync.dma_start(
                    out=xTq[1].rearrange("p (c n) -> p c n", n=NQ),
                    in_=xt[:, NQ:2 * NQ].rearrange("(c p) n -> p c n", p=128))
                nc.sync.dma_start(
                    out=xTq[2].rearrange("p (c n) -> p c n", n=NQ),
                    in_=xt[:, 2 * NQ:3 * NQ].rearrange("(c p) n -> p c n", p=128))
                nc.sync.dma_start(
                    out=xTq[3].rearrange("p (c n) -> p c n", n=NQ),
                    in_=xt[:, 3 * NQ:4 * NQ].rearrange("(c p) n -> p c n", p=128))
                load_w(wv_sb, wv)
                load_w(wq_sbh[0], wq[0:512, :])
                load_w(wq_sbh[1], wq[512:1024, :])

                # k for all m
                for nq in range(4):
                    for g in range(4):
                        ps = kqp.tile([128, 512], F32, tag="kq",
                                      name=f"k{nq}_{g}")
                        for c in range(8):
                            nc.tensor.matmul(
                                ps,
                                wk_sb[:, c * 512 + g * 128:
                                      c * 512 + (g + 1) * 128],
                                x_slice(nq, c, 0, 512),
                                start=(c == 0), stop=(c == 7))
                        n0 = nq * NQ
                        for hh in range(2):
                            h_, r0_ = 2 * g + hh, hh * D
                            nc.vector.tensor_scalar_add(
                                kTp[r0_:r0_ + D,
                                    h_ * N + n0: h_ * N + n0 + 512],
                                ps[r0_:r0_ + D, :],
                                bkc_sb[r0_:r0_ + D, g:g + 1])
                # v for all m
                for nq in range(4):
                    for ml in range(NQ // 128):
                        mc = nq * (NQ // 128) + ml
                        ps = vpp.tile([128, 512], F32, tag="v",
                                      name=f"v{nq}_{ml}")
                        for c in range(8):
                            nc.tensor.matmul(
                                ps,
                                x_slice(nq, c, ml * 128, (ml + 1) * 128),
                                wv_sb[:, c * 512:(c + 1) * 512],
                                start=(c == 0),
                                stop=(c == 7 and not with_biases))
                        if with_biases:
                            nc.tensor.matmul(ps, ones[0:1, 0:128],
                                             bv_sb[0:1, :],
                                             start=False, stop=True)
                        dst = v_sb[:, mc * HL * VS:(mc + 1) * HL * VS].rearrange(
                            "p (h e) -> p h e", e=VS)[:, :, 0:D]
                        nc.vector.tensor_copy(
                            dst, ps.rearrange("p (h e) -> p h e", e=D))
                # q group 0 (heads 0/1)
                for nq in range(4):
                    emit_q_group(nq, 0, kqp, "kq")

            # ---------- Phase B + C: flat attention stream + proj ----------
            # Flat stream over (head-unit u = nh*8+h, m-chunk mcc): per chunk
            # emit [av(u, mcc - lag), sc(u, mcc), exp(u, mcc)].  The av lag
            # staircase (doubles at hc 5/6, the last three m-chunks at the
            # next head's hc 0..2) gives the previous head's avs evacuation
            # time before its PSUM slot is reused.  q groups 1..3 and the
            # proj units are emitted into the stream; the priority-heap tile
            # scheduler slots them into PE gaps of the ACT-paced stream.
            # Softmax normalization: denominator row (av row 64) -> [128,8]
            # via sbuf-to-sbuf DMA -> DVE reciprocal -> row DMA -> stride-0
            # broadcast DMA to [64,1024] -> one DVE multiply into aoT.
            with tc.tile_pool(name="wppool", bufs=1) as wppool, \
                 tc.tile_pool(name="expp", bufs=8) as expp, \
                 tc.tile_pool(name="avsp", bufs=5) as avsp, \
                 tc.tile_pool(name="denp", bufs=4) as denp, \
                 tc.tile_pool(name="bcp", bufs=4) as bcp, \
                 tc.tile_pool(name="bpp", bufs=1) as bpp, \
                 tc.tile_pool(name="pout", bufs=3) as pout:
                wp_sb = wppool.tile([128, 4 * C], MDT, tag="wp", name="wp_sb")
                nc.sync.dma_start(
                    out=wp_sb.rearrange("p (g c) -> p g c", c=C),
                    in_=wp.rearrange("(g p) c -> p g c", p=128))
                bp_sb = bpp.tile([1, C], MDT, tag="bp")
                nc.sync.dma_start(out=bp_sb, in_=bp)

                with tc.tile_pool(name="scp", bufs=2, space="PSUM") as scp, \
                     tc.tile_pool(name="avp", bufs=1, space="PSUM") as avp, \
                     tc.tile_pool(name="pjp", bufs=2, space="PSUM") as pjp:

                    tails = []          # deferred normalization multiplies
                    proj_pending = []   # proj closures for a finished half
                    po_cur = {}         # nch -> pout tile awaiting 2nd jg

                    def emit_proj(nch, jg):
                        ps = pjp.tile([128, 512], F32, tag="pj",
                                      name=f"pj{nch}_{jg}")
                        for g in range(4):
                            nc.tensor.matmul(
                                ps,
                                aoT[:, g * N + nch * 128:
                                    g * N + (nch + 1) * 128],
                                wp_sb[:, g * C + jg * 512:
                                      g * C + jg * 512 + 512],
                                start=(g == 0),
                                stop=(g == 3 and not with_biases))
                        if with_biases:
                            nc.tensor.matmul(
                                ps, ones[0:1, 0:128],
                                bp_sb[0:1, jg * 512:(jg + 1) * 512],
                                start=False, stop=True)
                        if nch not in po_cur:
                            po_cur[nch] = pout.tile([128, C], F32, tag="po",
                                                    name=f"po{nch}")
                        po = po_cur[nch]
                        nc.vector.tensor_copy(
                            po[:, jg * 512:(jg + 1) * 512], ps)
                        if nch >= 14:
                            # final units: per-jg DMAs so the last transfer
                            # draining at kernel end is 256KB, not 512KB
                            nc.sync.dma_start(
                                out=out[nch * 128:(nch + 1) * 128,
                                        jg * 512:(jg + 1) * 512],
                                in_=po[:, jg * 512:(jg + 1) * 512])
                            if jg == 1:
                                del po_cur[nch]
                        elif jg == 1:
                            nc.sync.dma_start(
                                out=out[nch * 128:(nch + 1) * 128, :],
                                in_=po)
                            del po_cur[nch]

                    # per-head state
                    av_t = [None] * 16
                    ex_t = [[None] * 16 for _ in range(16)]

                    def emit_sc_exp(u, mcc):
                        nh, h = divmod(u, HL)
                        g, n0 = h // 2, nh * NHALF
                        sc = scp.tile([128, NHALF], F32, tag="sc",
                                      name=f"sc{u}_{mcc}")
                        for ngl in range(2):
                            nc.tensor.matmul(
                                sc[:, ngl * 512:(ngl + 1) * 512],
                                kTp[:, h * N + mcc * 128:
                                    h * N + (mcc + 1) * 128],
                                qT[:, g * N + n0 + ngl * 512:
                                   g * N + n0 + (ngl + 1) * 512],
                                start=True, stop=True)
                        ex = expp.tile([128, NHALF], BF16, tag="ex",
                                       name=f"ex{u}_{mcc}")
                        nc.scalar.activation(ex, sc, AFT.Exp, scale=SCALE)
                        ex_t[u][mcc] = ex

                    def emit_av(u, mcc):
                        h = u % HL
                        if mcc == 0:
                            av_t[u] = avp.tile([VS, NHALF], F32, tag="av",
                                               name=f"av{u}")
                        av = av_t[u]
                        ex = ex_t[u][mcc]
                        ex_t[u][mcc] = None
                        for ngl in range(2):
                            nc.tensor.matmul(
                                av[:, ngl * 512:(ngl + 1) * 512],
                                v_sb[:, (mcc * HL + h) * VS:
                                     (mcc * HL + h + 1) * VS],
                                ex[:, ngl * 512:(ngl + 1) * 512],
                                start=(mcc == 0), stop=(mcc == 15))

                    recips = []         # deferred reciprocal chains

                    def head_finish(u):
                        """avs evacuation for head u.  The reciprocal chain
                        is deferred a few chunks (so the DVE reciprocal's
                        den-DMA input has landed before it hits the in-order
                        DVE queue); the normalization multiply is deferred
                        four heads so its broadcast-DMA chain never delays
                        the in-order DVE queue.  The very last head uses a
                        PE broadcast matmul instead of the broadcast DMA to
                        shorten the end-of-kernel serial chain."""
                        nh, h = divmod(u, HL)
                        g, r0, n0 = h // 2, (h % 2) * D, nh * NHALF
                        last = (u == 15)
                        av = av_t[u]
                        avs = avsp.tile([VS, NHALF], MDT, tag="avs",
                                        name=f"avs{u}")
                        nc.vector.tensor_copy(avs, av[0:VS, :])
                        den = denp.tile([128, NHALF // 128], MDT,
                                        tag="den", name=f"den{u}")
                        nc.sync.dma_start(out=den, in_=avs[D:VS, :])
                        bcast = bcp.tile([D, NHALF], MDT, tag="bc",
                                         name=f"bc{u}")
                        rrow = denp.tile([1, NHALF], MDT, tag="rrow",
                                         name=f"rrow{u}")

                        def recip_chain():
                            rcp = denp.tile([128, NHALF // 128], MDT,
                                            tag="rcp", name=f"rcp{u}")
                            with nc.allow_low_precision(reason="softmax den"):
                                nc.vector.reciprocal(rcp, den)
                            nc.sync.dma_start(out=rrow, in_=rcp)
                            if not last:
                                # replicate the reciprocal row across 64
                                # partitions (64 x 2KB descriptors) so the
                                # normalization is one DVE multiply -- no PE
                                # broadcast matmul needed
                                nc.sync.dma_start(
                                    out=bcast,
                                    in_=rrow.unsqueeze(1).broadcast_to(
                                        [1, D, NHALF]))
                        recips.append(recip_chain)

                        def tail():
                            if last:
                                bc = scp.tile([D, NHALF], F32, tag="sc",
                                              name="bc15")
                                for ngl in range(2):
                                    nc.tensor.matmul(
                                        bc[:, ngl * 512:(ngl + 1) * 512],
                                        ones[0:1, 0:D],
                                        rrow[0:1, ngl * 512:(ngl + 1) * 512],
                                        start=True, stop=True)
                                src1 = bc
                            else:
                                src1 = bcast
                            nc.vector.tensor_mul(
                                aoT[r0:r0 + D,
                                    g * N + n0: g * N + n0 + NHALF],
                                avs[0:D, :], src1)
                        tails.append(tail)
                        if len(tails) > 3:
                            tails.pop(0)()

                    # av emission schedule per head chunk hc (0..15):
                    # doubles at hc 5/6, lag 3 afterwards, the last three
                    # m-chunks handled at the next head's hc 0..2.  The wide
                    # gap (hc2 boundary -> hc5 reuse) covers the previous
                    # head's avs-evacuation latency so the in-order PE queue
                    # never waits on the DVE.
                    def av_due(u, hc):
                        due = []
                        if u > 0 and hc <= 2:
                            due.append((u - 1, 13 + hc))
                        if hc == 5:
                            due.extend([(u, 0), (u, 1)])
                        elif hc == 6:
                            due.extend([(u, 2), (u, 3)])
                        elif hc >= 7:
                            due.append((u, hc - 3))
                        return due

                    for u in range(16):
                        for hc in range(16):
                            for (ua, mcc) in av_due(u, hc):
                                emit_av(ua, mcc)
                                if mcc == 15:
                                    head_finish(ua)
                                    # inject proj work for the finished half
                                    if u >= 12:
                                        for _ in range(4):
                                            if proj_pending:
                                                proj_pending.pop(0)()
                            if hc == 6 and recips:
                                recips.pop(0)()
                            emit_sc_exp(u, hc)
                        if u < 3:
                            # q projections for head-groups 1..3, scheduled
                            # into the stream's PE slack (pjp is idle here)
                            for nq in range(4):
                                emit_q_group(nq, u + 1, pjp, "pj")
                        if u == 7:
                            for nch in range(8):
                                for jg in range(2):
                                    proj_pending.append(
                                        lambda nch=nch, jg=jg:
                                            emit_proj(nch, jg))
                    # flush: remaining av chunks + last head's chain
                    emit_av(15, 13)
                    emit_av(15, 14)
                    emit_av(15, 15)
                    head_finish(15)
                    while recips:
                        recips.pop(0)()
                    while tails:
                        tails.pop(0)()
                    while proj_pending:
                        proj_pending.pop(0)()
                    for nch in range(8, 16):
                        for jg in range(2):
                            emit_proj(nch, jg)
    return nc


def _bf16(a):
    import ml_dtypes
    return np.ascontiguousarray(a).astype(ml_dtypes.bfloat16)


def shard_inputs(x, Wqkv, bqkv, Wproj, bproj):
    """Full inputs -> per-core in_maps. Core c: batch c//2, head-group c%2."""
    in_maps = []
    for core in range(N_CORES):
        b, hg = core // 2, core % 2
        s = hg * 512
        m = {
            "xt": _bf16(x[b].T),
            "wq": _bf16(Wqkv[:, s:s + 512]),
            "wk": _bf16(Wqkv[:, C + s: C + s + 512]),
            "wv": _bf16(Wqkv[:, 2 * C + s: 2 * C + s + 512]),
            "wp": _bf16(Wproj[s:s + 512, :]),
            "bqc": np.ascontiguousarray(bqkv[s:s + 512].reshape(4, 128).T),
            "bkc": np.ascontiguousarray(bqkv[C + s: C + s + 512].reshape(4, 128).T),
            "bv": _bf16(bqkv[2 * C + s: 2 * C + s + 512][None, :]),
            "bp": _bf16(
                (bproj if hg == 0 else np.zeros_like(bproj))[None, :]),
            "ones_row": _bf16(np.ones((1, 512), np.float32)),
        }
        in_maps.append(m)
    return in_maps


def unshard_output(results):
    """Per-core partial outputs -> full [4, N, C]."""
    outs = []
    for b in range(4):
        outs.append(results[2 * b]["out"] + results[2 * b + 1]["out"])
    return np.stack(outs, axis=0)


# revision 20
# speedup vs baseline: 1.1087x; 1.0036x over previous
"""Self-contained Trainium2 Bass kernel: 16-head self-attention (B=4, N=2048,
C=1024, fp32), SPMD across 8 NeuronCores.

Entry point: kernel(**inputs) -> np.ndarray matching the reference module
(qkv projection + scaled-dot-product softmax attention + output projection).
See build_nc() docstring for the kernel design.
"""
import numpy as np

_NC_CACHE = {}


def kernel(x, Wqkv, bqkv, Wproj, bproj):
    from concourse.bass_utils import run_bass_kernel_spmd
    x = np.asarray(x, dtype=np.float32)
    Wqkv = np.asarray(Wqkv, dtype=np.float32)
    bqkv = np.asarray(bqkv, dtype=np.float32)
    Wproj = np.asarray(Wproj, dtype=np.float32)
    bproj = np.asarray(bproj, dtype=np.float32)
    # the rank-1 bias accumulation steps are emitted only when any bias is
    # actually nonzero (they are exact zeros in this problem's inputs)
    wb = bool(np.any(bqkv) or np.any(bproj))
    if wb not in _NC_CACHE:
        nc = build_nc(with_biases=wb)
        split_excess_waits(nc)
        _NC_CACHE[wb] = nc
    nc = _NC_CACHE[wb]
    in_maps = shard_inputs(x, Wqkv, bqkv, Wproj, bproj)
    res = run_bass_kernel_spmd(nc, in_maps, core_ids=list(range(N_CORES)))
    return unshard_output(res.results).astype(np.float32)


# ======================================================================
# IR post-pass: this walrus build accepts at most one semaphore wait per
# instruction; overflow waits move onto chained NoOps just before the
# instruction on the same engine queue.
# ======================================================================

# Walrus TPB_CTRL codegen (Drain/NoOp lowering) accepts only 1 sync wait;
# regular engine instructions accept more (tested empirically).
CTRL_OPCODES = {"Drain", "NoOp", "EventSemaphore", "AllEngineBarrier"}

def split_excess_waits(nc, engine_max=1, ctrl_max=1):
    n_split = 0
    for f in nc.m.functions:
        for bb in f.blocks:
            insts = list(bb.instructions)
            out = []
            changed = False
            for inst in insts:
                si = inst.sync_info
                max_w = ctrl_max if inst.opcode in CTRL_OPCODES else engine_max
                if si is not None and si.on_wait and len(si.on_wait) > max_w:
                    waits = list(si.on_wait)
                    extra, keep = waits[max_w:], waits[:max_w]
                    for i in range(0, len(extra), ctrl_max):
                        nop = bass_rust.InstNoOp(
                            name=f"{inst.name}-wsplit{i}", ins=[], outs=[])
                        nop.engine = inst.engine
                        nop.sync_info = mybir.SyncInfo(
                            on_wait=extra[i:i + ctrl_max], on_update=[])
                        out.append(nop)
                        n_split += 1
                    inst.sync_info = mybir.SyncInfo(
                        on_wait=keep, on_update=list(si.on_update))
                    changed = True
                out.append(inst)
            if changed:
                bb.instructions = out
    return n_split


# ======================================================================
# Kernel proper
# ======================================================================
import bass_rust
import concourse.bass as bass
import concourse.tile as tile
import concourse.mybir as mybir


F32 = mybir.dt.float32
BF16 = mybir.dt.bfloat16

N = 2048        # sequence length
C = 1024        # embed dim
HL = 8          # heads handled per core
D = 64          # head dim
SCALE = D ** -0.5
NHALF = N // 2
NQ = N // 4     # x tile quarter
VS = D + 1      # v columns per head incl. ones column
N_CORES = 8

AFT = mybir.ActivationFunctionType
ALU = mybir.AluOpType


def build_nc(with_biases=True):
    MDT = BF16
    nc = bass.Bass("TRN2", target_bir_lowering=False, debug=False,
                   num_devices=N_CORES)
    xt = nc.dram_tensor("xt", [C, N], MDT, kind="ExternalInput").ap()
    wq = nc.dram_tensor("wq", [C, HL * D], MDT, kind="ExternalInput").ap()
    wk = nc.dram_tensor("wk", [C, HL * D], MDT, kind="ExternalInput").ap()
    wv = nc.dram_tensor("wv", [C, HL * D], MDT, kind="ExternalInput").ap()
    wp = nc.dram_tensor("wp", [HL * D, C], MDT, kind="ExternalInput").ap()
    bqc = nc.dram_tensor("bqc", [128, 4], F32, kind="ExternalInput").ap()
    bkc = nc.dram_tensor("bkc", [128, 4], F32, kind="ExternalInput").ap()
    bv = nc.dram_tensor("bv", [1, HL * D], MDT, kind="ExternalInput").ap()
    bp = nc.dram_tensor("bp", [1, C], MDT, kind="ExternalInput").ap()
    ones_row = nc.dram_tensor("ones_row", [1, 512], MDT, kind="ExternalInput").ap()
    out = nc.dram_tensor("out", [N, C], F32, kind="ExternalOutput").ap()

    with tile.TileContext(nc) as tc:
        with tc.tile_pool(name="consts", bufs=1) as consts, \
             tc.tile_pool(name="persist", bufs=1) as persist, \
             tc.tile_pool(name="big", bufs=1) as bigp, \
             tc.tile_pool(name="biases", bufs=1) as biasp, \
             tc.tile_pool(name="wqpool", bufs=2) as wqpool, \
             tc.tile_pool(name="xpool", bufs=4) as xpool:

            # persistent activation tiles (bf16 so the attention matmuls
            # get full 128x128 stationary tiles + fast weight loads):
            #   qT: pair-packed [2 heads' d x 128, pair-group x n]
            #   kTp: per-head [128, head x n] with the partner head's 64
            #        partition rows zeroed (K=128 contraction, zeros kill
            #        the partner-q contribution in the shared qT rhs)
            #   v_sb: per (m-chunk, head) [128, 65]: cols 0:64 v, col 64
            #        ones (softmax denominator)
            qT = persist.tile([128, 4 * N], BF16, tag="qT")
            kTp = persist.tile([128, HL * N], BF16, tag="kTp")
            v_sb = persist.tile([128, 16 * HL * VS], BF16, tag="v")
            nc.gpsimd.memset(kTp, 0.0)
            vview = v_sb.rearrange("p (m h e) -> p m h e", h=HL, e=VS)
            nc.gpsimd.memset(vview[:, :, :, D:D + 1], 1.0)
            aoT = bigp.tile([128, 4 * N], MDT, tag="big", name="aoT")

            # x quarters (quarter 0 as c-chunk half tiles so the first
            # matmuls wait on 0.5MB), wq halves, biases: these live through
            # phase B because the q projection for head-groups 1..3 runs
            # interleaved with the attention stream.
            xT0h = [xpool.tile([128, 4 * NQ], MDT, tag="xh",
                               name=f"xT0{i}") for i in range(2)]
            xTq = [None] + [xpool.tile([128, 8 * NQ], MDT, tag="x",
                                       name=f"xT{i}") for i in range(1, 4)]
            wq_sbh = [wqpool.tile([128, 4 * 512], MDT, tag="wh",
                                  name=f"wq_sb{i}") for i in range(2)]
            bqc_sb = biasp.tile([128, 4], F32, tag="bqc")
            bkc_sb = biasp.tile([128, 4], F32, tag="bkc")
            bv_sb = biasp.tile([1, HL * D], MDT, tag="bv")
            ones = consts.tile([1, 512], MDT, tag="ones")

            def wq_slice(c, g):
                return wq_sbh[c // 4][:, (c % 4) * 512 + g * 128:
                                      (c % 4) * 512 + (g + 1) * 128]

            def x_slice(nq, c, lo, hi):
                if nq == 0:
                    return xT0h[c // 4][:, (c % 4) * NQ + lo:
                                        (c % 4) * NQ + hi]
                return xTq[nq][:, c * NQ + lo:c * NQ + hi]

            def emit_q_group(nq, g, pool, tag):
                """One [128, 512] q output tile: contraction over C."""
                ps = pool.tile([128, 512], F32, tag=tag, name=f"q{nq}_{g}")
                for c in range(8):
                    nc.tensor.matmul(
                        ps, wq_slice(c, g), x_slice(nq, c, 0, 512),
                        start=(c == 0), stop=(c == 7))
                n0 = nq * NQ
                nc.vector.tensor_scalar_add(
                    qT[:, g * N + n0: g * N + n0 + 512],
                    ps, bqc_sb[:, g:g + 1])

            # ---------- Phase A: k and v for all m, then q group 0 ----------
            # (attention unit 0 only needs kTp, v_sb and qT group 0; the
            # remaining q groups are emitted inside the attention stream)
            with tc.tile_pool(name="wkv", bufs=2) as wkv, \
                 tc.tile_pool(name="kqp", bufs=2, space="PSUM") as kqp, \
                 tc.tile_pool(name="vpp", bufs=2, space="PSUM") as vpp:
                wk_sb = wkv.tile([128, 8 * 512], MDT, tag="w", name="wk_sb")
                wv_sb = wkv.tile([128, 8 * 512], MDT, tag="w", name="wv_sb")

                def load_w(dst, src):
                    nc.sync.dma_start(
                        out=dst.rearrange("p (c d) -> p c d", d=512),
                        in_=src.rearrange("(c p) d -> p c d", p=128))

                load_w(wk_sb, wk)
                nc.sync.dma_start(
                    out=xT0h[0].rearrange("p (c n) -> p c n", n=NQ),
                    in_=xt[0:512, 0:NQ].rearrange("(c p) n -> p c n", p=128))
                nc.sync.dma_start(
                    out=xT0h[1].rearrange("p (c n) -> p c n", n=NQ),
                    in_=xt[512:1024, 0:NQ].rearrange("(c p) n -> p c n", p=128))
                nc.sync.dma_start(out=ones, in_=ones_row)
                nc.sync.dma_start(out=bqc_sb, in_=bqc)
                nc.sync.dma_start(out=bkc_sb, in_=bkc)
                nc.sync.dma_start(out=bv_sb, in_=bv)
                nc.sync.dma_start(
                    out=xTq[1].rearrange("p (c n) -> p c n", n=NQ),
                    in_=xt[:, NQ:2 * NQ].rearrange("(c p) n -> p c n", p=128))
                nc.sync.dma_start(
                    out=xTq[2].rearrange("p (c n) -> p c n", n=NQ),
                    in_=xt[:, 2 * NQ:3 * NQ].rearrange("(c p) n -> p c n", p=128))
                nc.sync.dma_start(
                    out=xTq[3].rearrange("p (c n) -> p c n", n=NQ),
                    in_=xt[:, 3 * NQ:4 * NQ].rearrange("(c p) n -> p c n", p=128))
                load_w(wv_sb, wv)
                load_w(wq_sbh[0], wq[0:512, :])
                load_w(wq_sbh[1], wq[512:1024, :])

                # k for all m
                for nq in range(4):
                    for g in range(4):
                        ps = kqp.tile([128, 512], F32, tag="kq",
                                      name=f"k{nq}_{g}")
                        for c in range(8):
                            nc.tensor.matmul(
                                ps,
                                wk_sb[:, c * 512 + g * 128:
                                      c * 512 + (g + 1) * 128],
                                x_slice(nq, c, 0, 512),
                                start=(c == 0), stop=(c == 7))
                        n0 = nq * NQ
                        for hh in range(2):
                            h_, r0_ = 2 * g + hh, hh * D
                            nc.vector.tensor_scalar_add(
                                kTp[r0_:r0_ + D,
                                    h_ * N + n0: h_ * N + n0 + 512],
                                ps[r0_:r0_ + D, :],
                                bkc_sb[r0_:r0_ + D, g:g + 1])
                # v for all m
                for nq in range(4):
                    for ml in range(NQ // 128):
                        mc = nq * (NQ // 128) + ml
                        ps = vpp.tile([128, 512], F32, tag="v",
                                      name=f"v{nq}_{ml}")
                        for c in range(8):
                            nc.tensor.matmul(
                                ps,
                                x_slice(nq, c, ml * 128, (ml + 1) * 128),
                                wv_sb[:, c * 512:(c + 1) * 512],
                                start=(c == 0),
                                stop=(c == 7 and not with_biases))
                        if with_biases:
                            nc.tensor.matmul(ps, ones[0:1, 0:128],
                                             bv_sb[0:1, :],
                                             start=False, stop=True)
                        dst = v_sb[:, mc * HL * VS:(mc + 1) * HL * VS].rearrange(
                            "p (h e) -> p h e", e=VS)[:, :, 0:D]
                        nc.vector.tensor_copy(
                            dst, ps.rearrange("p (h e) -> p h e", e=D))
                # q group 0 (heads 0/1)
                for nq in range(4):
                    emit_q_group(nq, 0, kqp, "kq")

            # ---------- Phase B + C: flat attention stream + proj ----------
            # Flat stream over (head-unit u = nh*8+h, m-chunk mcc): per chunk
            # emit [av(u, mcc - lag), sc(u, mcc), exp(u, mcc)].  The av lag
            # staircase (doubles at hc 5/6, the last three m-chunks at the
            # next head's hc 0..2) gives the previous head's avs evacuation
            # time before its PSUM slot is reused.  q groups 1..3 and the
            # proj units are emitted into the stream; the priority-heap tile
            # scheduler slots them into PE gaps of the ACT-paced stream.
            # Softmax normalization: denominator row (av row 64) -> [128,8]
            # via sbuf-to-sbuf DMA -> DVE reciprocal -> row DMA -> stride-0
            # broadcast DMA to [64,1024] -> one DVE multiply into aoT.
            with tc.tile_pool(name="wppool", bufs=1) as wppool, \
                 tc.tile_pool(name="expp", bufs=8) as expp, \
                 tc.tile_pool(name="avsp", bufs=5) as avsp, \
                 tc.tile_pool(name="denp", bufs=4) as denp, \
                 tc.tile_pool(name="bpp", bufs=1) as bpp, \
                 tc.tile_pool(name="pout", bufs=3) as pout:
                wp_sb = wppool.tile([128, 4 * C], MDT, tag="wp", name="wp_sb")
                nc.sync.dma_start(
                    out=wp_sb.rearrange("p (g c) -> p g c", c=C),
                    in_=wp.rearrange("(g p) c -> p g c", p=128))
                bp_sb = bpp.tile([1, C], MDT, tag="bp")
                nc.sync.dma_start(out=bp_sb, in_=bp)

                with tc.tile_pool(name="scp", bufs=2, space="PSUM") as scp, \
                     tc.tile_pool(name="avp", bufs=1, space="PSUM") as avp, \
                     tc.tile_pool(name="pjp", bufs=2, space="PSUM") as pjp:

                    tails = []          # deferred normalization multiplies
                    proj_pending = []   # proj closures for a finished half
                    po_cur = {}         # nch -> pout tile awaiting 2nd jg

                    def emit_proj(nch, jg):
                        ps = pjp.tile([128, 512], F32, tag="pj",
                                      name=f"pj{nch}_{jg}")
                        for g in range(4):
                            nc.tensor.matmul(
                                ps,
                                aoT[:, g * N + nch * 128:
                                    g * N + (nch + 1) * 128],
                                wp_sb[:, g * C + jg * 512:
                                      g * C + jg * 512 + 512],
                                start=(g == 0),
                                stop=(g == 3 and not with_biases))
                        if with_biases:
                            nc.tensor.matmul(
                                ps, ones[0:1, 0:128],
                                bp_sb[0:1, jg * 512:(jg + 1) * 512],
                                start=False, stop=True)
                        if nch not in po_cur:
                            po_cur[nch] = pout.tile([128, C], F32, tag="po",
                                                    name=f"po{nch}")
                        po = po_cur[nch]
                        nc.vector.tensor_copy(
                            po[:, jg * 512:(jg + 1) * 512], ps)
                        if nch >= 14:
                            # final units: per-jg DMAs so the last transfer
                            # draining at kernel end is 256KB, not 512KB
                            nc.sync.dma_start(
                                out=out[nch * 128:(nch + 1) * 128,
                                        jg * 512:(jg + 1) * 512],
                                in_=po[:, jg * 512:(jg + 1) * 512])
                            if jg == 1:
                                del po_cur[nch]
                        elif jg == 1:
                            nc.sync.dma_start(
                                out=out[nch * 128:(nch + 1) * 128, :],
                                in_=po)
                            del po_cur[nch]

                    # per-head state
                    av_t = [None] * 16
                    ex_t = [[None] * 16 for _ in range(16)]

                    def emit_sc_exp(u, mcc):
                        nh, h = divmod(u, HL)
                        g, n0 = h // 2, nh * NHALF
                        sc = scp.tile([128, NHALF], F32, tag="sc",
                                      name=f"sc{u}_{mcc}")
                        for ngl in range(2):
                            nc.tensor.matmul(
                                sc[:, ngl * 512:(ngl + 1) * 512],
                                kTp[:, h * N + mcc * 128:
                                    h * N + (mcc + 1) * 128],
                                qT[:, g * N + n0 + ngl * 512:
                                   g * N + n0 + (ngl + 1) * 512],
                                start=True, stop=True)
                        ex = expp.tile([128, NHALF], BF16, tag="ex",
                                       name=f"ex{u}_{mcc}")
                        nc.scalar.activation(ex, sc, AFT.Exp, scale=SCALE)
                        ex_t[u][mcc] = ex

                    def emit_av(u, mcc):
                        h = u % HL
                        if mcc == 0:
                            av_t[u] = avp.tile([VS, NHALF], F32, tag="av",
                                               name=f"av{u}")
                        av = av_t[u]
                        ex = ex_t[u][mcc]
                        ex_t[u][mcc] = None
                        for ngl in range(2):
                            nc.tensor.matmul(
                                av[:, ngl * 512:(ngl + 1) * 512],
                                v_sb[:, (mcc * HL + h) * VS:
                                     (mcc * HL + h + 1) * VS],
                                ex[:, ngl * 512:(ngl + 1) * 512],
                                start=(mcc == 0), stop=(mcc == 15))

                    recips = []         # deferred reciprocal chains

                    def head_finish(u):
                        """avs evacuation for head u.  The reciprocal chain
                        is deferred a few chunks (so the DVE reciprocal's
                        den-DMA input has landed before it hits the in-order
                        DVE queue); the normalization multiply is deferred
                        four heads so its broadcast-DMA chain never delays
                        the in-order DVE queue.  The very last head uses a
                        PE broadcast matmul instead of the broadcast DMA to
                        shorten the end-of-kernel serial chain."""
                        nh, h = divmod(u, HL)
                        g, r0, n0 = h // 2, (h % 2) * D, nh * NHALF
                        av = av_t[u]
                        avs = avsp.tile([VS, NHALF], MDT, tag="avs",
                                        name=f"avs{u}")
                        nc.vector.tensor_copy(avs, av[0:VS, :])
                        den = denp.tile([128, NHALF // 128], MDT,
                                        tag="den", name=f"den{u}")
                        nc.sync.dma_start(out=den, in_=avs[D:VS, :])
                        rrow = denp.tile([1, NHALF], MDT, tag="rrow",
                                         name=f"rrow{u}")

                        def recip_chain():
                            rcp = denp.tile([128, NHALF // 128], MDT,
                                            tag="rcp", name=f"rcp{u}")
                            with nc.allow_low_precision(reason="softmax den"):
                                nc.vector.reciprocal(rcp, den)
                            nc.sync.dma_start(out=rrow, in_=rcp)
                        recips.append(recip_chain)

                        def tail():
                            # broadcast the reciprocal row across partitions
                            # on the PE (a stride-0 broadcast DMA hammers a
                            # single SBUF partition and arrives several us
                            # late, head-of-line-blocking the DVE queue)
                            bc = scp.tile([D, NHALF], F32, tag="sc",
                                          name=f"bcm{u}")
                            for ngl in range(2):
                                nc.tensor.matmul(
                                    bc[:, ngl * 512:(ngl + 1) * 512],
                                    ones[0:1, 0:D],
                                    rrow[0:1, ngl * 512:(ngl + 1) * 512],
                                    start=True, stop=True)
                            nc.vector.tensor_mul(
                                aoT[r0:r0 + D,
                                    g * N + n0: g * N + n0 + NHALF],
                                avs[0:D, :], bc)
                        tails.append(tail)
                        if len(tails) > 3:
                            tails.pop(0)()

                    # av emission schedule per head chunk hc (0..15):
                    # doubles at hc 5/6, lag 3 afterwards, the last three
                    # m-chunks handled at the next head's hc 0..2.  The wide
                    # gap (hc2 boundary -> hc5 reuse) covers the previous
                    # head's avs-evacuation latency so the in-order PE queue
                    # never waits on the DVE.
                    def av_due(u, hc):
                        due = []
                        if u > 0 and hc <= 2:
                            due.append((u - 1, 13 + hc))
                        if hc == 5:
                            due.extend([(u, 0), (u, 1)])
                        elif hc == 6:
                            due.extend([(u, 2), (u, 3)])
                        elif hc >= 7:
                            due.append((u, hc - 3))
                        return due

                    for u in range(16):
                        for hc in range(16):
                            for (ua, mcc) in av_due(u, hc):
                                emit_av(ua, mcc)
                                if mcc == 15:
                                    head_finish(ua)
                                    # inject proj work for the finished half
                                    if u >= 12:
                                        for _ in range(4):
                                            if proj_pending:
                                                proj_pending.pop(0)()
                            if hc == 6 and recips:
                                recips.pop(0)()
                            emit_sc_exp(u, hc)
                        if u < 3:
                            # q projections for head-groups 1..3, scheduled
                            # into the stream's PE slack (pjp is idle here)
                            for nq in range(4):
                                emit_q_group(nq, u + 1, pjp, "pj")
                        if u == 7:
                            for nch in range(8):
                                for jg in range(2):
                                    proj_pending.append(
                                        lambda nch=nch, jg=jg:
                                            emit_proj(nch, jg))
                    # flush: remaining av chunks + last head's chain
                    emit_av(15, 13)
                    emit_av(15, 14)
                    emit_av(15, 15)
                    head_finish(15)
                    while recips:
                        recips.pop(0)()
                    while tails:
                        tails.pop(0)()
                    while proj_pending:
                        proj_pending.pop(0)()
                    for nch in range(8, 16):
                        for jg in range(2):
                            emit_proj(nch, jg)
    return nc


def _bf16(a):
    import ml_dtypes
    return np.ascontiguousarray(a).astype(ml_dtypes.bfloat16)


def shard_inputs(x, Wqkv, bqkv, Wproj, bproj):
    """Full inputs -> per-core in_maps. Core c: batch c//2, head-group c%2."""
    in_maps = []
    for core in range(N_CORES):
        b, hg = core // 2, core % 2
        s = hg * 512
        m = {
            "xt": _bf16(x[b].T),
            "wq": _bf16(Wqkv[:, s:s + 512]),
            "wk": _bf16(Wqkv[:, C + s: C + s + 512]),
            "wv": _bf16(Wqkv[:, 2 * C + s: 2 * C + s + 512]),
            "wp": _bf16(Wproj[s:s + 512, :]),
            "bqc": np.ascontiguousarray(bqkv[s:s + 512].reshape(4, 128).T),
            "bkc": np.ascontiguousarray(bqkv[C + s: C + s + 512].reshape(4, 128).T),
            "bv": _bf16(bqkv[2 * C + s: 2 * C + s + 512][None, :]),
            "bp": _bf16(
                (bproj if hg == 0 else np.zeros_like(bproj))[None, :]),
            "ones_row": _bf16(np.ones((1, 512), np.float32)),
        }
        in_maps.append(m)
    return in_maps


def unshard_output(results):
    """Per-core partial outputs -> full [4, N, C]."""
    outs = []
    for b in range(4):
        outs.append(results[2 * b]["out"] + results[2 * b + 1]["out"])
    return np.stack(outs, axis=0)
